# revision 20
# baseline (speedup 1.0000x reference)
# Trainium2 Bass kernel for nn_MHAttentionMap (DETR-style attention map).
#
# Reference computation:
#   qp = q @ q_w.T + q_b                       [b, Q, 256]
#   kp = 1x1conv(k, k_w) + k_b                 [b, 256, H, W]
#   scores[b,q,n,s] = (qh*NORM) . kh           [b, Q, 8, H*W]
#   scores[mask] = -inf ; softmax over flattened (n, H, W) per (b, q)
#
# Sharding: 8 cores = (batch 0..3) x (query half 0..1); 150 queries/core.
# The softmax axis (heads x spatial) lives entirely on one core, so no
# collectives are needed.
#
# Masked-column compaction: masked (h,w) positions are exactly 0 in the
# output (exp(-inf)) and contribute nothing to the softmax sum, and the
# mask is known on the host per batch. The host gathers only the kept
# k-columns (padded to a static SPC with -30000-bias pad slots), the
# device computes scores/exp on SPC ~= S/2 columns, and the host
# scatters the compact output back to the full [.., 100, 100] layout via
# an XLA gather (masked positions read a zero column). This halves the
# dominant host<->device IO (k upload + attention-map download) and the
# device compute. If a mask ever keeps more than SPC columns, we fall
# back to the full-width program.
#
# Per-core device program (identical on all cores, different data):
#   - all projection inputs arrive as fp16 (halves H2D, 1-pass PE matmuls)
#   - qproj on PE -> qpT group tiles (fp16), NORM_FACT folded in
#   - kproj on PE (K=256) -> kp tiles [97/97/65, sp] fp16 grouped as
#     heads (0-2), (3-5), (6-7); last row of each = bias row
#     (0 / -30000, fp16, marshaled on host)
#   - scores: block-diagonal matmuls packing (heads-in-group x query-block)
#     into M<=126 with a ones-row in lhsT so the mask bias adds inside the
#     matmul (K = 32*hg + 1)
#   - exp on ACT directly from PSUM into per-pass fp16 buffers with
#     accum_out partial row sums; fold/unfold 0/1 matmuls (host constants)
#     reduce per-(head,query) sums into per-query totals and broadcast the
#     reciprocal back to the pass layout
#   - in-place DVE normalize, DMA out
#
# Host side: marshaling and the f16->f32 output conversion run through
# jax's CPU backend (multithreaded XLA) -- numpy's fp16 paths are ~40MB/s.
# Under axon the dispatch goes through a module-cached jax.jit of the
# bass_exec custom call (the stock run_bass_kernel_spmd rebuilds the jit
# and re-uploads 192MB of zero output buffers every call).

import numpy as np

import jax
import jax.numpy as jnp

import concourse.bacc as bacc
import concourse.bass as bass
import concourse.mybir as mybir
import concourse.tile as tile

try:
    from concourse._compat import axon_active
except ImportError:
    import os as _os

    def axon_active():
        return (bool(_os.environ.get("AXON_TERMINAL_JOB_NAME"))
                or _os.environ.get("AXON_H4_ENABLED") == "1")

QUERY_DIM = 256
HIDDEN = 256
NH = 8
HD = HIDDEN // NH  # 32
NORM_FACT = float(HIDDEN / NH) ** (-0.5)

B = 4
Q = 300
H = 100
W = 100
S = H * W  # 10000
NCORES = 8
QS = Q // 2  # 150 queries per core

# compact spatial width: P(Binomial(10000,1/2) > 5248) ~ 4e-7 per batch
SPC = 5248

# head groups: (#heads, first head)
HGROUPS = [(3, 0), (3, 3), (2, 6)]
# query rounds of 75, each split into blocks of (42, 33)
ROUND_Q = 75
QBLOCKS = [(0, 42), (42, 33)]

MASK_NEG = -30000.0

# fp16 output halves the dominant HBM write; verified <2e-3 rel err.
OUT_DTYPE = mybir.dt.float16

F32 = mybir.dt.float32
F16 = mybir.dt.float16
# NOTE: an fp8(E3M4) kproj datapath (k + k_w quantized, weights pre-scaled
# x32 out of the subnormal zone) was tried and REJECTED: CoreSim scale-rel
# error 2.6e-2 vs the 2e-2 gate (f16 path: 7.3e-4). Keep k/k_w in f16.


def _chunks(total, size):
    out = []
    off = 0
    while off < total:
        out.append((off, min(size, total - off)))
        off += size
    return out


def _fold_consts():
    # fold[qs*r + j, j] = 1 folds 3 stacked per-head rows into per-query;
    # unfold is its transpose (broadcast back to pass layout).
    consts = {}
    for qs in (42, 33):
        fold = np.zeros((3 * qs, qs), np.float32)
        for r in range(3):
            fold[qs * r + np.arange(qs), np.arange(qs)] = 1.0
        consts[f"fold{qs}"] = fold
        consts[f"unfold{qs}"] = np.ascontiguousarray(fold.T)
    return consts


def _emit(nc, tc, ctx, d, use_qbias, use_kbias, sp):
    """Emit the per-core program. d: dict of DRAM tensor handles."""
    consts = ctx.enter_context(tc.tile_pool(name="consts", bufs=1))
    persist = ctx.enter_context(tc.tile_pool(name="persist", bufs=1))
    work = ctx.enter_context(tc.tile_pool(name="work", bufs=3))
    small = ctx.enter_context(tc.tile_pool(name="small", bufs=4))
    psum = ctx.enter_context(tc.tile_pool(name="psum", bufs=2, space="PSUM"))

    n_parts = len(_chunks(sp, 2048))

    # ---- load constants ----
    qwT = []
    kwT = []
    for kb in range(2):
        t = consts.tile([128, 256], F16, tag=f"qwT{kb}", name=f"qwT{kb}")
        nc.sync.dma_start(out=t, in_=d["q_wT"][kb * 128:(kb + 1) * 128, :])
        qwT.append(t)
        t2 = consts.tile([128, 256], F16, tag=f"kwT{kb}", name=f"kwT{kb}")
        nc.sync.dma_start(out=t2, in_=d["k_wT"][kb * 128:(kb + 1) * 128, :])
        kwT.append(t2)
    qT = []
    for kb in range(2):
        t = consts.tile([128, QS], F16, tag=f"qT{kb}", name=f"qT{kb}")
        nc.sync.dma_start(out=t, in_=d["qT"][kb * 128:(kb + 1) * 128, :])
        qT.append(t)
    foldc = {}
    unfoldc = {}
    for qs in (42, 33):
        f = consts.tile([3 * qs, qs], F32, tag=f"fold{qs}", name=f"fold{qs}")
        nc.sync.dma_start(out=f, in_=d[f"fold{qs}"][:, :])
        foldc[qs] = f
        u = consts.tile([qs, 3 * qs], F32, tag=f"unfold{qs}", name=f"unfold{qs}")
        nc.sync.dma_start(out=u, in_=d[f"unfold{qs}"][:, :])
        unfoldc[qs] = u
    qbias_t = []
    kbias_t = []
    if use_qbias or use_kbias:
        for g, (hg, h0) in enumerate(HGROUPS):
            mg = 32 * hg
            if use_qbias:
                t = consts.tile([mg, 1], F32, tag=f"qb{g}", name=f"qb{g}")
                nc.sync.dma_start(out=t, in_=d[f"qbias{g}"][:, :])
                qbias_t.append(t)
            if use_kbias:
                t = consts.tile([mg, 1], F32, tag=f"kb{g}", name=f"kb{g}")
                nc.sync.dma_start(out=t, in_=d[f"kbias{g}"][:, :])
                kbias_t.append(t)

    # ---- qproj: qpT_g[g] [32*hg, 150] fp16 = (q_w @ q.T + q_b) * NORM ----
    qpT = []
    for g, (hg, h0) in enumerate(HGROUPS):
        mg = 32 * hg
        moff = 32 * h0
        ps = psum.tile([mg, QS], F32, tag="ps", name=f"qproj_ps{g}")
        for kb in range(2):
            nc.tensor.matmul(
                ps[0:mg, 0:QS],
                qwT[kb][:, moff:moff + mg],
                qT[kb][:, 0:QS],
                start=(kb == 0),
                stop=(kb == 1),
            )
        t = persist.tile([mg, QS], F16, tag=f"qpT{g}", name=f"qpT{g}")
        bias = qbias_t[g][0:mg, 0:1] if use_qbias else 0.0
        nc.scalar.activation(
            t[0:mg, 0:QS], ps[0:mg, 0:QS],
            mybir.ActivationFunctionType.Identity,
            bias=bias, scale=NORM_FACT,
        )
        qpT.append(t)

    # ---- block-diagonal lhsT staging tiles (both rounds) ----
    # stg[(r, g, qb)]: [K_g, M_p] fp16, K_g = 32*hg + 1 (ones row last),
    # block r' at rows 32r'..32r'+32, cols r'*qs..(r'+1)*qs.
    stg = {}
    for r in range(2):
        for g, (hg, h0) in enumerate(HGROUPS):
            kg = 32 * hg + 1
            for qb, (q0, qs) in enumerate(QBLOCKS):
                mp = hg * qs
                t = persist.tile([kg, 126], F16, tag=f"stg_{r}_{g}_{qb}",
                                 name=f"stg_{r}_{g}_{qb}")
                nc.vector.memset(t, 0.0)
                qa = r * ROUND_Q + q0
                for rr in range(hg):
                    nc.vector.tensor_copy(
                        t[32 * rr:32 * rr + 32, rr * qs:(rr + 1) * qs],
                        qpT[g][32 * rr:32 * rr + 32, qa:qa + qs],
                    )
                nc.vector.memset(t[kg - 1:kg, 0:mp], 1.0)
                stg[(r, g, qb)] = t

    # ---- kproj: kp[g] [32*hg + 1, sp] fp16, bias row last ----
    kp = []
    for g, (hg, h0) in enumerate(HGROUPS):
        kg = 32 * hg + 1
        t = persist.tile([kg, sp], F16, tag=f"kp{g}", name=f"kp{g}")
        nc.sync.dma_start(out=t[kg - 1:kg, :], in_=d["maskb"][0:1, :])
        kp.append(t)

    for c0, cw in _chunks(sp, 2048):
        kin = []
        for kb in range(2):
            t = work.tile([128, 2048], F16, tag=f"kin{kb}", bufs=2,
                          name=f"kin{kb}_{c0}")
            nc.sync.dma_start(out=t[:, 0:cw],
                              in_=d["k"][kb * 128:(kb + 1) * 128, c0:c0 + cw])
            kin.append(t)
        for g, (hg, h0) in enumerate(HGROUPS):
            mg = 32 * hg
            moff = 32 * h0
            ps = psum.tile([mg, 2048], F32, tag="ps", name=f"kproj_ps{g}_{c0}")
            for js, nw in _chunks(cw, 512):
                for kb in range(2):
                    nc.tensor.matmul(
                        ps[0:mg, js:js + nw],
                        kwT[kb][:, moff:moff + mg],
                        kin[kb][:, js:js + nw],
                        start=(kb == 0),
                        stop=(kb == 1),
                    )
            bias = kbias_t[g][0:mg, 0:1] if use_kbias else 0.0
            nc.scalar.activation(
                kp[g][0:mg, c0:c0 + cw], ps[0:mg, 0:cw],
                mybir.ActivationFunctionType.Identity,
                bias=bias, scale=1.0,
            )

    # ---- rounds: scores -> exp(+accum) -> sums -> normalize -> out ----
    for r in range(2):
        expb = {}
        sums = {}
        for qb, (q0, qs) in enumerate(QBLOCKS):
            t = small.tile([126, 3], F32, tag=f"sums_{r}_{qb}", bufs=1,
                           name=f"sums_{r}_{qb}")
            nc.vector.memset(t, 0.0)
            sums[qb] = t

        for qb, (q0, qs) in enumerate(QBLOCKS):
            for g, (hg, h0) in enumerate(HGROUPS):
                kg = 32 * hg + 1
                mp = hg * qs
                eb = work.tile([126, sp], F16, tag=f"expb_{g}_{qb}", bufs=1,
                               name=f"expb_{r}_{g}_{qb}")
                expb[(g, qb)] = eb
                parts = small.tile([126, n_parts], F32, tag="parts", bufs=3,
                                   name=f"parts_{r}_{g}_{qb}")
                lhs = stg[(r, g, qb)]
                for ci, (c0, cw) in enumerate(_chunks(sp, 2048)):
                    ps = psum.tile([126, 2048], F32, tag="ps",
                                   name=f"sc_ps_{r}_{g}_{qb}_{c0}")
                    for js, nw in _chunks(cw, 512):
                        nc.tensor.matmul(
                            ps[0:mp, js:js + nw],
                            lhs[0:kg, 0:mp],
                            kp[g][0:kg, c0 + js:c0 + js + nw],
                            start=True, stop=True,
                        )
                    nc.scalar.activation(
                        eb[0:mp, c0:c0 + cw], ps[0:mp, 0:cw],
                        mybir.ActivationFunctionType.Exp,
                        accum_out=parts[0:mp, ci:ci + 1],
                    )
                nc.vector.tensor_reduce(
                    sums[qb][0:mp, g:g + 1], parts[0:mp, 0:n_parts],
                    axis=mybir.AxisListType.X, op=mybir.AluOpType.add,
                )

        # per-query totals -> reciprocal -> broadcast to pass layout
        recP = {}
        for qb, (q0, qs) in enumerate(QBLOCKS):
            fps = psum.tile([qs, 3], F32, tag="ps", name=f"fold_ps_{r}_{qb}")
            nc.tensor.matmul(fps[0:qs, 0:3], foldc[qs][0:3 * qs, 0:qs],
                             sums[qb][0:3 * qs, 0:3], start=True, stop=True)
            tot = small.tile([qs, 1], F32, tag="tot", name=f"tot_{r}_{qb}")
            nc.vector.tensor_reduce(tot[0:qs, 0:1], fps[0:qs, 0:3],
                                    axis=mybir.AxisListType.X,
                                    op=mybir.AluOpType.add)
            rec = small.tile([qs, 1], F32, tag="rec", name=f"rec_{r}_{qb}")
            nc.vector.reciprocal(rec[0:qs, 0:1], tot[0:qs, 0:1])
            ups = psum.tile([3 * qs, 1], F32, tag="ps", name=f"unf_ps_{r}_{qb}")
            nc.tensor.matmul(ups[0:3 * qs, 0:1], unfoldc[qs][0:qs, 0:3 * qs],
                             rec[0:qs, 0:1], start=True, stop=True)
            rp = small.tile([126, 1], F32, tag=f"recP{qb}", bufs=2,
                            name=f"recP_{r}_{qb}")
            nc.vector.tensor_copy(rp[0:3 * qs, 0:1], ups[0:3 * qs, 0:1])
            recP[qb] = rp

        # normalize in place and write out
        out_r = d["out"][:].rearrange("q (h s) -> h q s", h=NH)
        for qb, (q0, qs) in enumerate(QBLOCKS):
            for g, (hg, h0) in enumerate(HGROUPS):
                mp = hg * qs
                eb = expb[(g, qb)]
                nc.vector.tensor_scalar_mul(
                    eb[0:mp, 0:sp], eb[0:mp, 0:sp], recP[qb][0:mp, 0:1],
                )
                qa = r * ROUND_Q + q0
                nc.sync.dma_start(
                    out=out_r[h0:h0 + hg, qa:qa + qs, :],
                    in_=eb[0:mp, 0:sp],
                )


_CACHED = {}


def _build(use_qbias, use_kbias, sp):
    key = (use_qbias, use_kbias, sp)
    if key in _CACHED:
        return _CACHED[key]
    nc = bacc.Bacc("TRN2", target_bir_lowering=False, debug=False)
    d = {}
    d["qT"] = nc.dram_tensor("qT", [256, QS], F16, kind="ExternalInput")
    d["k"] = nc.dram_tensor("k", [256, sp], F16, kind="ExternalInput")
    d["maskb"] = nc.dram_tensor("maskb", [1, sp], F16, kind="ExternalInput")
    d["q_wT"] = nc.dram_tensor("q_wT", [256, 256], F16, kind="ExternalInput")
    d["k_wT"] = nc.dram_tensor("k_wT", [256, 256], F16, kind="ExternalInput")
    for qs in (42, 33):
        d[f"fold{qs}"] = nc.dram_tensor(f"fold{qs}", [3 * qs, qs], F32,
                                        kind="ExternalInput")
        d[f"unfold{qs}"] = nc.dram_tensor(f"unfold{qs}", [qs, 3 * qs], F32,
                                          kind="ExternalInput")
    if use_qbias:
        for g, (hg, h0) in enumerate(HGROUPS):
            d[f"qbias{g}"] = nc.dram_tensor(f"qbias{g}", [32 * hg, 1], F32,
                                            kind="ExternalInput")
    if use_kbias:
        for g, (hg, h0) in enumerate(HGROUPS):
            d[f"kbias{g}"] = nc.dram_tensor(f"kbias{g}", [32 * hg, 1], F32,
                                            kind="ExternalInput")
    d["out"] = nc.dram_tensor("out", [QS, NH * sp], OUT_DTYPE,
                              kind="ExternalOutput")
    from contextlib import ExitStack
    with tile.TileContext(nc) as tc:
        with ExitStack() as ctx:
            _emit(nc, tc, ctx, d, use_qbias, use_kbias, sp)
    nc.compile()
    _CACHED[key] = nc
    return nc


# ---------------------------------------------------------------------------
# Host marshaling (jax CPU backend -- numpy fp16 conversions are ~40MB/s).
# ---------------------------------------------------------------------------

_CPU_FNS = {}


def _cpu_fn(name):
    if name in _CPU_FNS:
        return _CPU_FNS[name]

    def marshal_qw(q, q_w, k_w):
        # per-core qT: [4,300,256] -> [4,2,256,150] -> [2048,150] f16
        qT = q.reshape(B, 2, QS, QUERY_DIM).transpose(0, 1, 3, 2)
        qT = qT.reshape(NCORES * QUERY_DIM, QS).astype(jnp.float16)
        qwT = jnp.tile(q_w.T.astype(jnp.float16), (NCORES, 1))
        kwT = jnp.tile(k_w.T.astype(jnp.float16), (NCORES, 1))
        return qT, qwT, kwT

    def marshal_k_full(k, mask):
        # per-core k: [4,256,100,100] -> dup x2 -> [2048,10000] f8
        k8 = k.reshape(B, 1, QUERY_DIM, S).astype(jnp.float16)
        k8 = jnp.broadcast_to(k8, (B, 2, QUERY_DIM, S))
        k8 = k8.reshape(NCORES * QUERY_DIM, S)
        mb = jnp.where(mask.reshape(B, 1, S), jnp.float16(MASK_NEG),
                       jnp.float16(0.0))
        mb = jnp.broadcast_to(mb, (B, 2, S)).reshape(NCORES, S)
        return k8, mb

    def marshal_k_compact(k, idx):
        # gather kept columns: k [4,256,10000] f32, idx [4,SPC] int32
        kc = jnp.take_along_axis(k.reshape(B, QUERY_DIM, S), idx[:, None, :],
                                 axis=2).astype(jnp.float16)
        kc = jnp.broadcast_to(kc[:, None], (B, 2, QUERY_DIM, SPC))
        return kc.reshape(NCORES * QUERY_DIM, SPC)

    def convert_full(o16):
        # [1200, 80000] f16 -> [4,300,8,100,100] f32
        return o16.astype(jnp.float32).reshape(B, Q, NH, H, W)

    def convert_compact(o16, gidx):
        # o16 [1200, 8*SPC] f16, gidx [4, S] int32 (SPC = zero dummy)
        v = o16.reshape(B, Q, NH, SPC)
        v = jnp.concatenate([v, jnp.zeros((B, Q, NH, 1), jnp.float16)],
                            axis=3)
        full = jnp.take_along_axis(v, gidx[:, None, None, :], axis=3)
        return full.astype(jnp.float32).reshape(B, Q, NH, H, W)

    def convert_compact_shard(o16, gidx):
        # o16 [QS, 8*SPC] f16 (one core), gidx [S] int32
        v = o16.reshape(QS, NH, SPC)
        v = jnp.concatenate([v, jnp.zeros((QS, NH, 1), jnp.float16)], axis=2)
        full = jnp.take_along_axis(v, gidx[None, None, :], axis=2)
        return full.astype(jnp.float32).reshape(QS, NH, H, W)

    def convert_full_shard(o16):
        return o16.astype(jnp.float32).reshape(QS, NH, H, W)

    fns = {"marshal_qw": marshal_qw, "marshal_k_full": marshal_k_full,
           "marshal_k_compact": marshal_k_compact,
           "convert_full": convert_full, "convert_compact": convert_compact,
           "convert_compact_shard": convert_compact_shard,
           "convert_full_shard": convert_full_shard}
    for n, f in fns.items():
        _CPU_FNS[n] = jax.jit(f, backend="cpu")
    return _CPU_FNS[name]


def _prepare(q, k, mask, q_w, q_b, k_w, k_b):
    """Marshal inputs. Returns (nc, global_input_dict, postprocess).

    Global arrays stack the 8 per-core shards on axis 0 (core order =
    (batch, query-half) lexicographic), matching shard_map's P("core")."""
    use_qbias = bool(np.any(q_b != 0))
    use_kbias = bool(np.any(k_b != 0))

    mask = np.asarray(mask).reshape(B, S)
    counts = (~mask).sum(axis=1)
    compact = counts.max() <= SPC

    qT, qwT, kwT = (np.asarray(a) for a in
                    _cpu_fn("marshal_qw")(q, q_w, k_w))
    g = {"qT": qT, "q_wT": qwT, "k_wT": kwT}

    if compact:
        idx = np.zeros((B, SPC), np.int32)
        gidx = np.full((B, S), SPC, np.int32)
        maskb = np.full((B, SPC), np.float16(MASK_NEG))
        for b in range(B):
            kept = np.nonzero(~mask[b])[0]
            n = len(kept)
            idx[b, :n] = kept
            idx[b, n:] = kept[-1] if n else 0
            gidx[b, kept] = np.arange(n, dtype=np.int32)
            maskb[b, :n] = np.float16(0.0)
        g["k"] = np.asarray(_cpu_fn("marshal_k_compact")(
            k.reshape(B, QUERY_DIM, S), idx))
        g["maskb"] = np.broadcast_to(
            maskb[:, None], (B, 2, SPC)).reshape(NCORES, SPC).copy()
        sp = SPC
        conv = _cpu_fn("convert_compact")
        conv_shard = _cpu_fn("convert_compact_shard")

        def post(o16):
            return np.asarray(conv(o16, gidx))

        def post_shard(c, part):
            return np.asarray(conv_shard(part, gidx[c // 2]))
    else:
        k16, mb = _cpu_fn("marshal_k_full")(k, mask.reshape(B, H, W))
        g["k"] = np.asarray(k16)
        g["maskb"] = np.asarray(mb)
        sp = S
        conv = _cpu_fn("convert_full")
        conv_shard = _cpu_fn("convert_full_shard")

        def post(o16):
            return np.asarray(conv(o16))

        def post_shard(c, part):
            return np.asarray(conv_shard(part))

    for fname, arr in _fold_consts().items():
        g[fname] = np.tile(arr, (NCORES, 1))
    if use_qbias:
        qb_scaled = (q_b.astype(np.float32) * NORM_FACT).reshape(256, 1)
        for gi, (hg, h0) in enumerate(HGROUPS):
            g[f"qbias{gi}"] = np.tile(
                np.ascontiguousarray(qb_scaled[32 * h0:32 * h0 + 32 * hg]),
                (NCORES, 1))
    if use_kbias:
        kb_col = k_b.astype(np.float32).reshape(256, 1)
        for gi, (hg, h0) in enumerate(HGROUPS):
            g[f"kbias{gi}"] = np.tile(
                np.ascontiguousarray(kb_col[32 * h0:32 * h0 + 32 * hg]),
                (NCORES, 1))

    nc = _build(use_qbias, use_kbias, sp)
    return nc, g, post, post_shard


def make_in_maps(q, k, mask, q_w, q_b, k_w, k_b):
    """Per-core input dicts + postprocess (sim / native-path use)."""
    nc, g, post, _ = _prepare(q, k, mask, q_w, q_b, k_w, k_b)
    in_maps = []
    for c in range(NCORES):
        m = {}
        for name, arr in g.items():
            rows = arr.shape[0] // NCORES
            m[name] = np.ascontiguousarray(arr[c * rows:(c + 1) * rows])
        in_maps.append(m)
    return nc, in_maps, post


# ---------------------------------------------------------------------------
# Execution: cached jit over the bass_exec custom call (axon PJRT path).
# ---------------------------------------------------------------------------

_RUNNERS = {}


def _get_runner(nc):
    key = id(nc)
    if key in _RUNNERS:
        return _RUNNERS[key]

    from concourse.bass2jax import (_bass_exec_p, install_neuronx_cc_hook,
                                    partition_id_tensor)
    from jax.sharding import Mesh, PartitionSpec, NamedSharding
    try:
        from jax.experimental.shard_map import shard_map
    except ImportError:
        shard_map = jax.shard_map

    install_neuronx_cc_hook()

    partition_name = (nc.partition_id_tensor.name
                      if nc.partition_id_tensor else None)
    in_names, out_names, out_avals = [], [], []
    for alloc in nc.m.functions[0].allocations:
        if not isinstance(alloc, mybir.MemoryLocationSet):
            continue
        name = alloc.memorylocations[0].name
        if alloc.kind == "ExternalInput":
            if name != partition_name:
                in_names.append(name)
        elif alloc.kind == "ExternalOutput":
            out_names.append(name)
            out_avals.append(jax.core.ShapedArray(
                tuple(alloc.tensor_shape), mybir.dt.np(alloc.dtype)))
    n_params = len(in_names)
    bind_names = tuple(in_names + ([partition_name] if partition_name else []))

    devices = jax.devices()[:NCORES]
    mesh = Mesh(np.asarray(devices), ("core",))
    P = PartitionSpec

    def _body(*args):
        operands = list(args)
        if partition_name:
            operands.append(partition_id_tensor())
        outs = _bass_exec_p.bind(
            *operands, out_avals=tuple(out_avals), in_names=bind_names,
            out_names=tuple(out_names), lowering_input_output_aliases=(),
            sim_require_finite=True, sim_require_nnan=True, nc=nc)
        return tuple(outs)

    sh = NamedSharding(mesh, P("core"))

    def _make_jit():
        return jax.jit(shard_map(
            _body, mesh=mesh, in_specs=(P("core"),) * n_params,
            out_specs=(P("core"),) * len(out_names), check_rep=False))

    sharded = None
    try:
        # AOT-compile with the bass effect suppressed: C++ fast-path
        # dispatch (the effectful path re-enters Python every call).
        from concourse.bass2jax import fast_dispatch_compile
        arg_structs = []
        for name in in_names:
            alloc_shape = None
            for alloc in nc.m.functions[0].allocations:
                if (isinstance(alloc, mybir.MemoryLocationSet)
                        and alloc.memorylocations[0].name == name):
                    alloc_shape = (NCORES * alloc.tensor_shape[0],
                                   *alloc.tensor_shape[1:])
                    dt = mybir.dt.np(alloc.dtype)
                    break
            arg_structs.append(
                jax.ShapeDtypeStruct(alloc_shape, dt, sharding=sh))
        sharded = fast_dispatch_compile(
            lambda: _make_jit().lower(*arg_structs).compile())
    except Exception:
        sharded = None
    if sharded is None:
        sharded = _make_jit()
    runner = {"fn": sharded, "in_names": in_names, "out_names": out_names,
              "sharding": sh, "const_cache": {}}
    _RUNNERS[key] = runner
    return runner


def _run_axon(nc, global_inputs, post_shard):
    """Dispatch via the cached jit; fetch + convert shards pipelined
    (converts hide inside the serialized tunnel transfers)."""
    from concurrent.futures import ThreadPoolExecutor
    r = _get_runner(nc)
    # device-cache the fold/unfold constants (identical every call)
    args = []
    for name in r["in_names"]:
        arr = global_inputs[name]
        if name.startswith(("fold", "unfold")):
            darr = r["const_cache"].get(name)
            if darr is None:
                darr = jax.device_put(np.asarray(arr), r["sharding"])
                r["const_cache"][name] = darr
            args.append(darr)
        else:
            args.append(arr)
    out = r["fn"](*args)[0]
    shards = sorted(out.addressable_shards,
                    key=lambda s: s.index[0].start or 0)
    final = np.empty((B, Q, NH, H, W), np.float32)

    def work(c):
        part = np.asarray(shards[c].data)
        b, qh = c // 2, c % 2
        final[b, qh * QS:(qh + 1) * QS] = post_shard(c, part)

    with ThreadPoolExecutor(4) as ex:
        list(ex.map(work, range(NCORES)))
    return final


def kernel(q, k, mask, q_w, q_b, k_w, k_b):
    import os
    nc, g, post, post_shard = _prepare(q, k, mask, q_w, q_b, k_w, k_b)
    if axon_active() and not os.environ.get("BASS_TRACE"):
        return _run_axon(nc, g, post_shard)
    from concourse.bass_utils import run_bass_kernel_spmd
    in_maps = []
    for c in range(NCORES):
        m = {}
        for name, arr in g.items():
            rows = arr.shape[0] // NCORES
            m[name] = np.ascontiguousarray(arr[c * rows:(c + 1) * rows])
        in_maps.append(m)
    res = run_bass_kernel_spmd(nc, in_maps, core_ids=list(range(NCORES)))
    out16 = np.concatenate([r["out"] for r in res.results], axis=0)
    return post(out16)


# revision 26
# speedup vs baseline: 1.0741x; 1.0741x over previous
# Trainium2 Bass kernel for nn_MHAttentionMap (DETR-style attention map).
#
# Reference computation:
#   qp = q @ q_w.T + q_b                       [b, Q, 256]
#   kp = 1x1conv(k, k_w) + k_b                 [b, 256, H, W]
#   scores[b,q,n,s] = (qh*NORM) . kh           [b, Q, 8, H*W]
#   scores[mask] = -inf ; softmax over flattened (n, H, W) per (b, q)
#
# Sharding: 8 cores = (batch 0..3) x (query half 0..1); 150 queries/core.
# The softmax axis (heads x spatial) lives entirely on one core, so no
# collectives are needed.
#
# Masked-column compaction: masked (h,w) positions are exactly 0 in the
# output (exp(-inf)) and contribute nothing to the softmax sum, and the
# mask is known on the host per batch. The host gathers only the kept
# k-columns (padded to a static SPC with -30000-bias pad slots), the
# device computes scores/exp on SPC ~= S/2 columns, and the host
# scatters the compact output back to the full [.., 100, 100] layout via
# an XLA gather (masked positions read a zero column). This halves the
# dominant host<->device IO (k upload + attention-map download) and the
# device compute. If a mask ever keeps more than SPC columns, we fall
# back to the full-width program.
#
# Per-core device program (identical on all cores, different data):
#   - all projection inputs arrive as fp16 (halves H2D, 1-pass PE matmuls)
#   - qproj on PE -> qpT group tiles (fp16), NORM_FACT folded in
#   - kproj on PE (K=256) -> kp tiles [97/97/65, sp] fp16 grouped as
#     heads (0-2), (3-5), (6-7); last row of each = bias row
#     (0 / -30000, fp16, marshaled on host)
#   - scores: block-diagonal matmuls packing (heads-in-group x query-block)
#     into M<=126 with a ones-row in lhsT so the mask bias adds inside the
#     matmul (K = 32*hg + 1)
#   - exp on ACT directly from PSUM into per-pass fp16 buffers with
#     accum_out partial row sums; fold/unfold 0/1 matmuls (host constants)
#     reduce per-(head,query) sums into per-query totals and broadcast the
#     reciprocal back to the pass layout
#   - in-place DVE normalize, DMA out
#
# Host side: marshaling and the f16->f32 output conversion run through
# jax's CPU backend (multithreaded XLA) -- numpy's fp16 paths are ~40MB/s.
# Under axon the dispatch goes through a module-cached jax.jit of the
# bass_exec custom call (the stock run_bass_kernel_spmd rebuilds the jit
# and re-uploads 192MB of zero output buffers every call).

import numpy as np

import jax
import jax.numpy as jnp

import concourse.bacc as bacc
import concourse.bass as bass
import concourse.mybir as mybir
import concourse.tile as tile

try:
    from concourse._compat import axon_active
except ImportError:
    import os as _os

    def axon_active():
        return (bool(_os.environ.get("AXON_TERMINAL_JOB_NAME"))
                or _os.environ.get("AXON_H4_ENABLED") == "1")

QUERY_DIM = 256
HIDDEN = 256
NH = 8
HD = HIDDEN // NH  # 32
NORM_FACT = float(HIDDEN / NH) ** (-0.5)

B = 4
Q = 300
H = 100
W = 100
S = H * W  # 10000
NCORES = 8
QS = Q // 2  # 150 queries per core

# compact spatial width: P(Binomial(10000,1/2) > 5248) ~ 4e-7 per batch
SPC = 5248

# head groups: (#heads, first head)
HGROUPS = [(3, 0), (3, 3), (2, 6)]
# query rounds of 75, each split into blocks of (42, 33)
ROUND_Q = 75
QBLOCKS = [(0, 42), (42, 33)]

MASK_NEG = -30000.0

# fp16 output halves the dominant HBM write; verified <2e-3 rel err.
OUT_DTYPE = mybir.dt.float16

F32 = mybir.dt.float32
F16 = mybir.dt.float16
# NOTE: an fp8(E3M4) kproj datapath (k + k_w quantized, weights pre-scaled
# x32 out of the subnormal zone) was tried and REJECTED: CoreSim scale-rel
# error 2.6e-2 vs the 2e-2 gate (f16 path: 7.3e-4). Keep k/k_w in f16.


def _chunks(total, size):
    out = []
    off = 0
    while off < total:
        out.append((off, min(size, total - off)))
        off += size
    return out


def _fold_consts():
    # fold[qs*r + j, j] = 1 folds 3 stacked per-head rows into per-query;
    # unfold is its transpose (broadcast back to pass layout).
    consts = {}
    for qs in (42, 33):
        fold = np.zeros((3 * qs, qs), np.float32)
        for r in range(3):
            fold[qs * r + np.arange(qs), np.arange(qs)] = 1.0
        consts[f"fold{qs}"] = fold
        consts[f"unfold{qs}"] = np.ascontiguousarray(fold.T)
    return consts


def _emit(nc, tc, ctx, d, use_qbias, use_kbias, sp, half_k=False):
    """Emit the per-core program. d: dict of DRAM tensor handles.

    half_k: each core of a (batch) pair uploads and projects only half of
    the k columns; the fp16 kp halves are exchanged with a paired
    AllGather through DRAM bounce tiles (input IO -44%)."""
    consts = ctx.enter_context(tc.tile_pool(name="consts", bufs=1))
    persist = ctx.enter_context(tc.tile_pool(name="persist", bufs=1))
    work = ctx.enter_context(tc.tile_pool(name="work", bufs=3))
    small = ctx.enter_context(tc.tile_pool(name="small", bufs=4))
    psum = ctx.enter_context(tc.tile_pool(name="psum", bufs=2, space="PSUM"))
    if half_k:
        dram = ctx.enter_context(tc.tile_pool(name="dram", bufs=1,
                                              space="DRAM"))

    n_parts = len(_chunks(sp, 2048))

    # ---- load constants ----
    qwT = []
    kwT = []
    for kb in range(2):
        t = consts.tile([128, 256], F16, tag=f"qwT{kb}", name=f"qwT{kb}")
        nc.sync.dma_start(out=t, in_=d["q_wT"][kb * 128:(kb + 1) * 128, :])
        qwT.append(t)
        t2 = consts.tile([128, 256], F16, tag=f"kwT{kb}", name=f"kwT{kb}")
        nc.sync.dma_start(out=t2, in_=d["k_wT"][kb * 128:(kb + 1) * 128, :])
        kwT.append(t2)
    qT = []
    for kb in range(2):
        t = consts.tile([128, QS], F16, tag=f"qT{kb}", name=f"qT{kb}")
        nc.sync.dma_start(out=t, in_=d["qT"][kb * 128:(kb + 1) * 128, :])
        qT.append(t)
    foldc = {}
    unfoldc = {}
    for qs in (42, 33):
        f = consts.tile([3 * qs, qs], F32, tag=f"fold{qs}", name=f"fold{qs}")
        nc.sync.dma_start(out=f, in_=d[f"fold{qs}"][:, :])
        foldc[qs] = f
        u = consts.tile([qs, 3 * qs], F32, tag=f"unfold{qs}", name=f"unfold{qs}")
        nc.sync.dma_start(out=u, in_=d[f"unfold{qs}"][:, :])
        unfoldc[qs] = u
    qbias_t = []
    kbias_t = []
    if use_qbias or use_kbias:
        for g, (hg, h0) in enumerate(HGROUPS):
            mg = 32 * hg
            if use_qbias:
                t = consts.tile([mg, 1], F32, tag=f"qb{g}", name=f"qb{g}")
                nc.sync.dma_start(out=t, in_=d[f"qbias{g}"][:, :])
                qbias_t.append(t)
            if use_kbias:
                t = consts.tile([mg, 1], F32, tag=f"kb{g}", name=f"kb{g}")
                nc.sync.dma_start(out=t, in_=d[f"kbias{g}"][:, :])
                kbias_t.append(t)

    # ---- qproj: qpT_g[g] [32*hg, 150] fp16 = (q_w @ q.T + q_b) * NORM ----
    qpT = []
    for g, (hg, h0) in enumerate(HGROUPS):
        mg = 32 * hg
        moff = 32 * h0
        ps = psum.tile([mg, QS], F32, tag="ps", name=f"qproj_ps{g}")
        for kb in range(2):
            nc.tensor.matmul(
                ps[0:mg, 0:QS],
                qwT[kb][:, moff:moff + mg],
                qT[kb][:, 0:QS],
                start=(kb == 0),
                stop=(kb == 1),
            )
        t = persist.tile([mg, QS], F16, tag=f"qpT{g}", name=f"qpT{g}")
        bias = qbias_t[g][0:mg, 0:1] if use_qbias else 0.0
        nc.scalar.activation(
            t[0:mg, 0:QS], ps[0:mg, 0:QS],
            mybir.ActivationFunctionType.Identity,
            bias=bias, scale=NORM_FACT,
        )
        qpT.append(t)

    # ---- block-diagonal lhsT staging tiles (both rounds) ----
    # stg[(r, g, qb)]: [K_g, M_p] fp16, K_g = 32*hg + 1 (ones row last),
    # block r' at rows 32r'..32r'+32, cols r'*qs..(r'+1)*qs.
    stg = {}
    for r in range(2):
        for g, (hg, h0) in enumerate(HGROUPS):
            kg = 32 * hg + 1
            for qb, (q0, qs) in enumerate(QBLOCKS):
                mp = hg * qs
                t = persist.tile([kg, 126], F16, tag=f"stg_{r}_{g}_{qb}",
                                 name=f"stg_{r}_{g}_{qb}")
                nc.vector.memset(t, 0.0)
                qa = r * ROUND_Q + q0
                for rr in range(hg):
                    nc.vector.tensor_copy(
                        t[32 * rr:32 * rr + 32, rr * qs:(rr + 1) * qs],
                        qpT[g][32 * rr:32 * rr + 32, qa:qa + qs],
                    )
                nc.vector.memset(t[kg - 1:kg, 0:mp], 1.0)
                stg[(r, g, qb)] = t

    # ---- kproj: kp[g] [32*hg + 1, sp] fp16, bias row last ----
    kp = []
    for g, (hg, h0) in enumerate(HGROUPS):
        kg = 32 * hg + 1
        t = persist.tile([kg, sp], F16, tag=f"kp{g}", name=f"kp{g}")
        nc.sync.dma_start(out=t[kg - 1:kg, :], in_=d["maskb"][0:1, :])
        kp.append(t)

    # columns this core projects (all of sp, or its half under half_k)
    kw_cols = sp // 2 if half_k else sp
    if half_k:
        kph = dram.tile([256, kw_cols], F16, tag="kph", name="kph")
        kpg = dram.tile([512, kw_cols], F16, tag="kpg", name="kpg")
        kp_half = []
        for g, (hg, h0) in enumerate(HGROUPS):
            mg = 32 * hg
            t = persist.tile([mg, kw_cols], F16, tag=f"kph{g}",
                             name=f"kph{g}")
            kp_half.append(t)

    for c0, cw in _chunks(kw_cols, 2048):
        kin = []
        for kb in range(2):
            t = work.tile([128, 2048], F16, tag=f"kin{kb}", bufs=2,
                          name=f"kin{kb}_{c0}")
            nc.sync.dma_start(out=t[:, 0:cw],
                              in_=d["k"][kb * 128:(kb + 1) * 128, c0:c0 + cw])
            kin.append(t)
        for g, (hg, h0) in enumerate(HGROUPS):
            mg = 32 * hg
            moff = 32 * h0
            ps = psum.tile([mg, 2048], F32, tag="ps", name=f"kproj_ps{g}_{c0}")
            for js, nw in _chunks(cw, 512):
                for kb in range(2):
                    nc.tensor.matmul(
                        ps[0:mg, js:js + nw],
                        kwT[kb][:, moff:moff + mg],
                        kin[kb][:, js:js + nw],
                        start=(kb == 0),
                        stop=(kb == 1),
                    )
            bias = kbias_t[g][0:mg, 0:1] if use_kbias else 0.0
            dst = kp_half[g] if half_k else kp[g]
            nc.scalar.activation(
                dst[0:mg, c0:c0 + cw], ps[0:mg, 0:cw],
                mybir.ActivationFunctionType.Identity,
                bias=bias, scale=1.0,
            )

    if half_k:
        # bounce kp halves to DRAM, AllGather within (even, odd) pairs,
        # reload both halves into the full-width kp tiles
        for g, (hg, h0) in enumerate(HGROUPS):
            mg = 32 * hg
            moff = 32 * h0
            nc.gpsimd.dma_start(kph[moff:moff + mg, :], kp_half[g][0:mg, :])
        nc.gpsimd.collective_compute(
            "AllGather", mybir.AluOpType.bypass,
            replica_groups=[[0, 1], [2, 3], [4, 5], [6, 7]],
            ins=[kph.opt()], outs=[kpg.opt()],
        )
        for g, (hg, h0) in enumerate(HGROUPS):
            mg = 32 * hg
            moff = 32 * h0
            nc.gpsimd.dma_start(kp[g][0:mg, 0:kw_cols],
                                kpg[moff:moff + mg, :])
            nc.gpsimd.dma_start(kp[g][0:mg, kw_cols:sp],
                                kpg[256 + moff:256 + moff + mg, :])

    # ---- rounds: scores -> exp(+accum) -> sums -> normalize -> out ----
    for r in range(2):
        expb = {}
        sums = {}
        for qb, (q0, qs) in enumerate(QBLOCKS):
            t = small.tile([126, 3], F32, tag=f"sums_{r}_{qb}", bufs=1,
                           name=f"sums_{r}_{qb}")
            nc.vector.memset(t, 0.0)
            sums[qb] = t

        for qb, (q0, qs) in enumerate(QBLOCKS):
            for g, (hg, h0) in enumerate(HGROUPS):
                kg = 32 * hg + 1
                mp = hg * qs
                eb = work.tile([126, sp], F16, tag=f"expb_{g}_{qb}", bufs=1,
                               name=f"expb_{r}_{g}_{qb}")
                expb[(g, qb)] = eb
                parts = small.tile([126, n_parts], F32, tag="parts", bufs=3,
                                   name=f"parts_{r}_{g}_{qb}")
                lhs = stg[(r, g, qb)]
                for ci, (c0, cw) in enumerate(_chunks(sp, 2048)):
                    ps = psum.tile([126, 2048], F32, tag="ps",
                                   name=f"sc_ps_{r}_{g}_{qb}_{c0}")
                    for js, nw in _chunks(cw, 512):
                        nc.tensor.matmul(
                            ps[0:mp, js:js + nw],
                            lhs[0:kg, 0:mp],
                            kp[g][0:kg, c0 + js:c0 + js + nw],
                            start=True, stop=True,
                        )
                    nc.scalar.activation(
                        eb[0:mp, c0:c0 + cw], ps[0:mp, 0:cw],
                        mybir.ActivationFunctionType.Exp,
                        accum_out=parts[0:mp, ci:ci + 1],
                    )
                nc.vector.tensor_reduce(
                    sums[qb][0:mp, g:g + 1], parts[0:mp, 0:n_parts],
                    axis=mybir.AxisListType.X, op=mybir.AluOpType.add,
                )

        # per-query totals -> reciprocal -> broadcast to pass layout
        recP = {}
        for qb, (q0, qs) in enumerate(QBLOCKS):
            fps = psum.tile([qs, 3], F32, tag="ps", name=f"fold_ps_{r}_{qb}")
            nc.tensor.matmul(fps[0:qs, 0:3], foldc[qs][0:3 * qs, 0:qs],
                             sums[qb][0:3 * qs, 0:3], start=True, stop=True)
            tot = small.tile([qs, 1], F32, tag="tot", name=f"tot_{r}_{qb}")
            nc.vector.tensor_reduce(tot[0:qs, 0:1], fps[0:qs, 0:3],
                                    axis=mybir.AxisListType.X,
                                    op=mybir.AluOpType.add)
            rec = small.tile([qs, 1], F32, tag="rec", name=f"rec_{r}_{qb}")
            nc.vector.reciprocal(rec[0:qs, 0:1], tot[0:qs, 0:1])
            ups = psum.tile([3 * qs, 1], F32, tag="ps", name=f"unf_ps_{r}_{qb}")
            nc.tensor.matmul(ups[0:3 * qs, 0:1], unfoldc[qs][0:qs, 0:3 * qs],
                             rec[0:qs, 0:1], start=True, stop=True)
            rp = small.tile([126, 1], F32, tag=f"recP{qb}", bufs=2,
                            name=f"recP_{r}_{qb}")
            nc.vector.tensor_copy(rp[0:3 * qs, 0:1], ups[0:3 * qs, 0:1])
            recP[qb] = rp

        # normalize in place and write out
        out_r = d["out"][:].rearrange("q (h s) -> h q s", h=NH)
        for qb, (q0, qs) in enumerate(QBLOCKS):
            for g, (hg, h0) in enumerate(HGROUPS):
                mp = hg * qs
                eb = expb[(g, qb)]
                nc.vector.tensor_scalar_mul(
                    eb[0:mp, 0:sp], eb[0:mp, 0:sp], recP[qb][0:mp, 0:1],
                )
                qa = r * ROUND_Q + q0
                nc.sync.dma_start(
                    out=out_r[h0:h0 + hg, qa:qa + qs, :],
                    in_=eb[0:mp, 0:sp],
                )


_CACHED = {}


def _build(use_qbias, use_kbias, sp, half_k=False):
    key = (use_qbias, use_kbias, sp, half_k)
    if key in _CACHED:
        return _CACHED[key]
    nc = bacc.Bacc("TRN2", target_bir_lowering=False, debug=False)
    d = {}
    d["qT"] = nc.dram_tensor("qT", [256, QS], F16, kind="ExternalInput")
    d["k"] = nc.dram_tensor("k", [256, sp // 2 if half_k else sp], F16,
                            kind="ExternalInput")
    d["maskb"] = nc.dram_tensor("maskb", [1, sp], F16, kind="ExternalInput")
    d["q_wT"] = nc.dram_tensor("q_wT", [256, 256], F16, kind="ExternalInput")
    d["k_wT"] = nc.dram_tensor("k_wT", [256, 256], F16, kind="ExternalInput")
    for qs in (42, 33):
        d[f"fold{qs}"] = nc.dram_tensor(f"fold{qs}", [3 * qs, qs], F32,
                                        kind="ExternalInput")
        d[f"unfold{qs}"] = nc.dram_tensor(f"unfold{qs}", [qs, 3 * qs], F32,
                                          kind="ExternalInput")
    if use_qbias:
        for g, (hg, h0) in enumerate(HGROUPS):
            d[f"qbias{g}"] = nc.dram_tensor(f"qbias{g}", [32 * hg, 1], F32,
                                            kind="ExternalInput")
    if use_kbias:
        for g, (hg, h0) in enumerate(HGROUPS):
            d[f"kbias{g}"] = nc.dram_tensor(f"kbias{g}", [32 * hg, 1], F32,
                                            kind="ExternalInput")
    d["out"] = nc.dram_tensor("out", [QS, NH * sp], OUT_DTYPE,
                              kind="ExternalOutput")
    from contextlib import ExitStack
    with tile.TileContext(nc) as tc:
        with ExitStack() as ctx:
            _emit(nc, tc, ctx, d, use_qbias, use_kbias, sp, half_k=half_k)
    nc.compile()
    _CACHED[key] = nc
    return nc


# ---------------------------------------------------------------------------
# Host marshaling (jax CPU backend -- numpy fp16 conversions are ~40MB/s).
# ---------------------------------------------------------------------------

_CPU_FNS = {}


def _cpu_fn(name):
    if name in _CPU_FNS:
        return _CPU_FNS[name]

    def marshal_qw(q, q_w, k_w):
        # per-core qT: [4,300,256] -> [4,2,256,150] -> [2048,150] f16
        qT = q.reshape(B, 2, QS, QUERY_DIM).transpose(0, 1, 3, 2)
        qT = qT.reshape(NCORES * QUERY_DIM, QS).astype(jnp.float16)
        qwT = jnp.tile(q_w.T.astype(jnp.float16), (NCORES, 1))
        kwT = jnp.tile(k_w.T.astype(jnp.float16), (NCORES, 1))
        return qT, qwT, kwT

    def marshal_k_full(k, mask):
        # per-core k: [4,256,100,100] -> dup x2 -> [2048,10000] f8
        k8 = k.reshape(B, 1, QUERY_DIM, S).astype(jnp.float16)
        k8 = jnp.broadcast_to(k8, (B, 2, QUERY_DIM, S))
        k8 = k8.reshape(NCORES * QUERY_DIM, S)
        mb = jnp.where(mask.reshape(B, 1, S), jnp.float16(MASK_NEG),
                       jnp.float16(0.0))
        mb = jnp.broadcast_to(mb, (B, 2, S)).reshape(NCORES, S)
        return k8, mb

    def marshal_k_compact(k, idx):
        # gather kept columns: k [4,256,10000] f32, idx [4,SPC] int32.
        # column-half split: core (b,0) gets cols 0:SPC/2, (b,1) the rest
        # (recombined on-device by the paired kp AllGather)
        kc = jnp.take_along_axis(k.reshape(B, QUERY_DIM, S), idx[:, None, :],
                                 axis=2).astype(jnp.float16)
        kc = kc.reshape(B, QUERY_DIM, 2, SPC // 2).transpose(0, 2, 1, 3)
        return kc.reshape(NCORES * QUERY_DIM, SPC // 2)

    def convert_full(o16):
        # [1200, 80000] f16 -> [4,300,8,100,100] f32
        return o16.astype(jnp.float32).reshape(B, Q, NH, H, W)

    def convert_compact(o16, gidx):
        # o16 [1200, 8*SPC] f16, gidx [4, S] int32 (SPC = zero dummy)
        v = o16.reshape(B, Q, NH, SPC)
        v = jnp.concatenate([v, jnp.zeros((B, Q, NH, 1), jnp.float16)],
                            axis=3)
        full = jnp.take_along_axis(v, gidx[:, None, None, :], axis=3)
        return full.astype(jnp.float32).reshape(B, Q, NH, H, W)

    def convert_compact_shard(o16, gidx):
        # o16 [QS, 8*SPC] f16 (one core), gidx [S] int32
        v = o16.reshape(QS, NH, SPC)
        v = jnp.concatenate([v, jnp.zeros((QS, NH, 1), jnp.float16)], axis=2)
        full = jnp.take_along_axis(v, gidx[None, None, :], axis=2)
        return full.astype(jnp.float32).reshape(QS, NH, H, W)

    def convert_full_shard(o16):
        return o16.astype(jnp.float32).reshape(QS, NH, H, W)

    fns = {"marshal_qw": marshal_qw, "marshal_k_full": marshal_k_full,
           "marshal_k_compact": marshal_k_compact,
           "convert_full": convert_full, "convert_compact": convert_compact,
           "convert_compact_shard": convert_compact_shard,
           "convert_full_shard": convert_full_shard}
    for n, f in fns.items():
        _CPU_FNS[n] = jax.jit(f, backend="cpu")
    return _CPU_FNS[name]


def _prepare(q, k, mask, q_w, q_b, k_w, k_b):
    """Marshal inputs. Returns (nc, global_input_dict, postprocess).

    Global arrays stack the 8 per-core shards on axis 0 (core order =
    (batch, query-half) lexicographic), matching shard_map's P("core")."""
    use_qbias = bool(np.any(q_b != 0))
    use_kbias = bool(np.any(k_b != 0))

    mask = np.asarray(mask).reshape(B, S)
    counts = (~mask).sum(axis=1)
    compact = counts.max() <= SPC

    qT, qwT, kwT = (np.asarray(a) for a in
                    _cpu_fn("marshal_qw")(q, q_w, k_w))
    g = {"qT": qT, "q_wT": qwT, "k_wT": kwT}

    if compact:
        idx = np.zeros((B, SPC), np.int32)
        gidx = np.full((B, S), SPC, np.int32)
        maskb = np.full((B, SPC), np.float16(MASK_NEG))
        for b in range(B):
            kept = np.nonzero(~mask[b])[0]
            n = len(kept)
            idx[b, :n] = kept
            idx[b, n:] = kept[-1] if n else 0
            gidx[b, kept] = np.arange(n, dtype=np.int32)
            maskb[b, :n] = np.float16(0.0)
        g["k"] = np.asarray(_cpu_fn("marshal_k_compact")(
            k.reshape(B, QUERY_DIM, S), idx))
        g["maskb"] = np.broadcast_to(
            maskb[:, None], (B, 2, SPC)).reshape(NCORES, SPC).copy()
        sp = SPC
        conv = _cpu_fn("convert_compact")
        conv_shard = _cpu_fn("convert_compact_shard")

        def post(o16):
            return np.asarray(conv(o16, gidx))

        def post_shard(c, part):
            return np.asarray(conv_shard(part, gidx[c // 2]))
    else:
        k16, mb = _cpu_fn("marshal_k_full")(k, mask.reshape(B, H, W))
        g["k"] = np.asarray(k16)
        g["maskb"] = np.asarray(mb)
        sp = S
        conv = _cpu_fn("convert_full")
        conv_shard = _cpu_fn("convert_full_shard")

        def post(o16):
            return np.asarray(conv(o16))

        def post_shard(c, part):
            return np.asarray(conv_shard(part))

    for fname, arr in _fold_consts().items():
        g[fname] = np.tile(arr, (NCORES, 1))
    if use_qbias:
        qb_scaled = (q_b.astype(np.float32) * NORM_FACT).reshape(256, 1)
        for gi, (hg, h0) in enumerate(HGROUPS):
            g[f"qbias{gi}"] = np.tile(
                np.ascontiguousarray(qb_scaled[32 * h0:32 * h0 + 32 * hg]),
                (NCORES, 1))
    if use_kbias:
        kb_col = k_b.astype(np.float32).reshape(256, 1)
        for gi, (hg, h0) in enumerate(HGROUPS):
            g[f"kbias{gi}"] = np.tile(
                np.ascontiguousarray(kb_col[32 * h0:32 * h0 + 32 * hg]),
                (NCORES, 1))

    nc = _build(use_qbias, use_kbias, sp, half_k=compact)
    return nc, g, post, post_shard


def make_in_maps(q, k, mask, q_w, q_b, k_w, k_b):
    """Per-core input dicts + postprocess (sim / native-path use)."""
    nc, g, post, _ = _prepare(q, k, mask, q_w, q_b, k_w, k_b)
    in_maps = []
    for c in range(NCORES):
        m = {}
        for name, arr in g.items():
            rows = arr.shape[0] // NCORES
            m[name] = np.ascontiguousarray(arr[c * rows:(c + 1) * rows])
        in_maps.append(m)
    return nc, in_maps, post


# ---------------------------------------------------------------------------
# Execution: cached jit over the bass_exec custom call (axon PJRT path).
# ---------------------------------------------------------------------------

_RUNNERS = {}


def _get_runner(nc):
    key = id(nc)
    if key in _RUNNERS:
        return _RUNNERS[key]

    from concourse.bass2jax import (_bass_exec_p, install_neuronx_cc_hook,
                                    partition_id_tensor)
    from jax.sharding import Mesh, PartitionSpec, NamedSharding
    try:
        from jax.experimental.shard_map import shard_map
    except ImportError:
        shard_map = jax.shard_map

    install_neuronx_cc_hook()

    partition_name = (nc.partition_id_tensor.name
                      if nc.partition_id_tensor else None)
    in_names, out_names, out_avals = [], [], []
    for alloc in nc.m.functions[0].allocations:
        if not isinstance(alloc, mybir.MemoryLocationSet):
            continue
        name = alloc.memorylocations[0].name
        if alloc.kind == "ExternalInput":
            if name != partition_name:
                in_names.append(name)
        elif alloc.kind == "ExternalOutput":
            out_names.append(name)
            out_avals.append(jax.core.ShapedArray(
                tuple(alloc.tensor_shape), mybir.dt.np(alloc.dtype)))
    n_params = len(in_names)
    bind_names = tuple(in_names + ([partition_name] if partition_name else []))

    devices = jax.devices()[:NCORES]
    mesh = Mesh(np.asarray(devices), ("core",))
    P = PartitionSpec

    def _body(*args):
        operands = list(args)
        if partition_name:
            operands.append(partition_id_tensor())
        outs = _bass_exec_p.bind(
            *operands, out_avals=tuple(out_avals), in_names=bind_names,
            out_names=tuple(out_names), lowering_input_output_aliases=(),
            sim_require_finite=True, sim_require_nnan=True, nc=nc)
        return tuple(outs)

    sh = NamedSharding(mesh, P("core"))

    def _make_jit():
        return jax.jit(shard_map(
            _body, mesh=mesh, in_specs=(P("core"),) * n_params,
            out_specs=(P("core"),) * len(out_names), check_rep=False))

    sharded = None
    try:
        # AOT-compile with the bass effect suppressed: C++ fast-path
        # dispatch (the effectful path re-enters Python every call).
        from concourse.bass2jax import fast_dispatch_compile
        arg_structs = []
        for name in in_names:
            alloc_shape = None
            for alloc in nc.m.functions[0].allocations:
                if (isinstance(alloc, mybir.MemoryLocationSet)
                        and alloc.memorylocations[0].name == name):
                    alloc_shape = (NCORES * alloc.tensor_shape[0],
                                   *alloc.tensor_shape[1:])
                    dt = mybir.dt.np(alloc.dtype)
                    break
            arg_structs.append(
                jax.ShapeDtypeStruct(alloc_shape, dt, sharding=sh))
        sharded = fast_dispatch_compile(
            lambda: _make_jit().lower(*arg_structs).compile())
    except Exception:
        sharded = None
    if sharded is None:
        sharded = _make_jit()
    runner = {"fn": sharded, "in_names": in_names, "out_names": out_names,
              "sharding": sh, "const_cache": {}}
    _RUNNERS[key] = runner
    return runner


def _run_axon(nc, global_inputs, post_shard):
    """Dispatch via the cached jit; fetch + convert shards pipelined
    (converts hide inside the serialized tunnel transfers)."""
    from concurrent.futures import ThreadPoolExecutor
    r = _get_runner(nc)
    # device-cache the fold/unfold constants (identical every call)
    args = []
    for name in r["in_names"]:
        arr = global_inputs[name]
        if name.startswith(("fold", "unfold")):
            darr = r["const_cache"].get(name)
            if darr is None:
                darr = jax.device_put(np.asarray(arr), r["sharding"])
                r["const_cache"][name] = darr
            args.append(darr)
        else:
            args.append(arr)
    out = r["fn"](*args)[0]
    shards = sorted(out.addressable_shards,
                    key=lambda s: s.index[0].start or 0)
    final = np.empty((B, Q, NH, H, W), np.float32)

    def work(c):
        part = np.asarray(shards[c].data)
        b, qh = c // 2, c % 2
        final[b, qh * QS:(qh + 1) * QS] = post_shard(c, part)

    with ThreadPoolExecutor(4) as ex:
        list(ex.map(work, range(NCORES)))
    return final


def kernel(q, k, mask, q_w, q_b, k_w, k_b):
    import os
    nc, g, post, post_shard = _prepare(q, k, mask, q_w, q_b, k_w, k_b)
    if axon_active() and not os.environ.get("BASS_TRACE"):
        return _run_axon(nc, g, post_shard)
    from concourse.bass_utils import run_bass_kernel_spmd
    in_maps = []
    for c in range(NCORES):
        m = {}
        for name, arr in g.items():
            rows = arr.shape[0] // NCORES
            m[name] = np.ascontiguousarray(arr[c * rows:(c + 1) * rows])
        in_maps.append(m)
    res = run_bass_kernel_spmd(nc, in_maps, core_ids=list(range(NCORES)))
    out16 = np.concatenate([r["out"] for r in res.results], axis=0)
    return post(out16)


# revision 27
# speedup vs baseline: 1.1023x; 1.0262x over previous
# Trainium2 Bass kernel for nn_MHAttentionMap (DETR-style attention map).
#
# Reference computation:
#   qp = q @ q_w.T + q_b                       [b, Q, 256]
#   kp = 1x1conv(k, k_w) + k_b                 [b, 256, H, W]
#   scores[b,q,n,s] = (qh*NORM) . kh           [b, Q, 8, H*W]
#   scores[mask] = -inf ; softmax over flattened (n, H, W) per (b, q)
#
# Sharding: 8 cores = (batch 0..3) x (query half 0..1); 150 queries/core.
# The softmax axis (heads x spatial) lives entirely on one core. The only
# cross-core exchange is a paired AllGather of the fp16 kproj halves:
# both cores of a batch pair need the same kp, so each uploads + projects
# half of the (compacted) k columns and gathers the other half on-device
# (input IO -44% vs duplicating k, output identical bit-for-bit).
#
# Masked-column compaction: masked (h,w) positions are exactly 0 in the
# output (exp(-inf)) and contribute nothing to the softmax sum, and the
# mask is known on the host per batch. The host gathers only the kept
# k-columns (padded to a static SPC with -30000-bias pad slots), the
# device computes scores/exp on SPC ~= S/2 columns, and the host
# scatters the compact output back to the full [.., 100, 100] layout via
# an XLA gather (masked positions read a zero column). This halves the
# dominant host<->device IO (k upload + attention-map download) and the
# device compute. If a mask ever keeps more than SPC columns, we fall
# back to the full-width program.
#
# Per-core device program (identical on all cores, different data):
#   - all projection inputs arrive as fp16 (halves H2D, 1-pass PE matmuls)
#   - qproj on PE -> qpT group tiles (fp16), NORM_FACT folded in
#   - kproj on PE (K=256) -> kp tiles [97/97/65, sp] fp16 grouped as
#     heads (0-2), (3-5), (6-7); last row of each = bias row
#     (0 / -30000, fp16, marshaled on host)
#   - scores: block-diagonal matmuls packing (heads-in-group x query-block)
#     into M<=126 with a ones-row in lhsT so the mask bias adds inside the
#     matmul (K = 32*hg + 1)
#   - exp on ACT directly from PSUM into per-pass fp16 buffers with
#     accum_out partial row sums; fold/unfold 0/1 matmuls (host constants)
#     reduce per-(head,query) sums into per-query totals and broadcast the
#     reciprocal back to the pass layout
#   - in-place DVE normalize, DMA out
#
# Host side: marshaling and the f16->f32 output conversion run through
# jax's CPU backend (multithreaded XLA) -- numpy's fp16 paths are ~40MB/s.
# Under axon the dispatch goes through a module-cached jax.jit of the
# bass_exec custom call (the stock run_bass_kernel_spmd rebuilds the jit
# and re-uploads 192MB of zero output buffers every call).

import numpy as np

import jax
import jax.numpy as jnp

import concourse.bacc as bacc
import concourse.bass as bass
import concourse.mybir as mybir
import concourse.tile as tile

try:
    from concourse._compat import axon_active
except ImportError:
    import os as _os

    def axon_active():
        return (bool(_os.environ.get("AXON_TERMINAL_JOB_NAME"))
                or _os.environ.get("AXON_H4_ENABLED") == "1")

QUERY_DIM = 256
HIDDEN = 256
NH = 8
HD = HIDDEN // NH  # 32
NORM_FACT = float(HIDDEN / NH) ** (-0.5)

B = 4
Q = 300
H = 100
W = 100
S = H * W  # 10000
NCORES = 8
QS = Q // 2  # 150 queries per core

# compact spatial width: P(Binomial(10000,1/2) > 5248) ~ 4e-7 per batch
SPC = 5248

# head groups: (#heads, first head)
HGROUPS = [(3, 0), (3, 3), (2, 6)]
# query rounds of 75, each split into blocks of (42, 33)
ROUND_Q = 75
QBLOCKS = [(0, 42), (42, 33)]

MASK_NEG = -30000.0

# fp16 output halves the dominant HBM write; verified <2e-3 rel err.
OUT_DTYPE = mybir.dt.float16

F32 = mybir.dt.float32
F16 = mybir.dt.float16
# NOTE: an fp8(E3M4) kproj datapath (k + k_w quantized, weights pre-scaled
# x32 out of the subnormal zone) was tried and REJECTED: CoreSim scale-rel
# error 2.6e-2 vs the 2e-2 gate (f16 path: 7.3e-4). Keep k/k_w in f16.


def _chunks(total, size):
    out = []
    off = 0
    while off < total:
        out.append((off, min(size, total - off)))
        off += size
    return out


def _fold_consts():
    # fold[qs*r + j, j] = 1 folds 3 stacked per-head rows into per-query;
    # unfold is its transpose (broadcast back to pass layout).
    consts = {}
    for qs in (42, 33):
        fold = np.zeros((3 * qs, qs), np.float32)
        for r in range(3):
            fold[qs * r + np.arange(qs), np.arange(qs)] = 1.0
        consts[f"fold{qs}"] = fold
        consts[f"unfold{qs}"] = np.ascontiguousarray(fold.T)
    return consts


def _emit(nc, tc, ctx, d, use_qbias, use_kbias, sp, half_k=False):
    """Emit the per-core program. d: dict of DRAM tensor handles.

    half_k: each core of a (batch) pair uploads and projects only half of
    the k columns; the fp16 kp halves are exchanged with a paired
    AllGather through DRAM bounce tiles (input IO -44%)."""
    consts = ctx.enter_context(tc.tile_pool(name="consts", bufs=1))
    persist = ctx.enter_context(tc.tile_pool(name="persist", bufs=1))
    work = ctx.enter_context(tc.tile_pool(name="work", bufs=3))
    small = ctx.enter_context(tc.tile_pool(name="small", bufs=4))
    psum = ctx.enter_context(tc.tile_pool(name="psum", bufs=2, space="PSUM"))
    if half_k:
        dram = ctx.enter_context(tc.tile_pool(name="dram", bufs=1,
                                              space="DRAM"))

    n_parts = len(_chunks(sp, 2048))

    # ---- load constants ----
    qwT = []
    kwT = []
    for kb in range(2):
        t = consts.tile([128, 256], F16, tag=f"qwT{kb}", name=f"qwT{kb}")
        nc.sync.dma_start(out=t, in_=d["q_wT"][kb * 128:(kb + 1) * 128, :])
        qwT.append(t)
        t2 = consts.tile([128, 256], F16, tag=f"kwT{kb}", name=f"kwT{kb}")
        nc.sync.dma_start(out=t2, in_=d["k_wT"][kb * 128:(kb + 1) * 128, :])
        kwT.append(t2)
    qT = []
    for kb in range(2):
        t = consts.tile([128, QS], F16, tag=f"qT{kb}", name=f"qT{kb}")
        nc.sync.dma_start(out=t, in_=d["qT"][kb * 128:(kb + 1) * 128, :])
        qT.append(t)
    foldc = {}
    unfoldc = {}
    for qs in (42, 33):
        f = consts.tile([3 * qs, qs], F32, tag=f"fold{qs}", name=f"fold{qs}")
        nc.sync.dma_start(out=f, in_=d[f"fold{qs}"][:, :])
        foldc[qs] = f
        u = consts.tile([qs, 3 * qs], F32, tag=f"unfold{qs}", name=f"unfold{qs}")
        nc.sync.dma_start(out=u, in_=d[f"unfold{qs}"][:, :])
        unfoldc[qs] = u
    qbias_t = []
    kbias_t = []
    if use_qbias or use_kbias:
        for g, (hg, h0) in enumerate(HGROUPS):
            mg = 32 * hg
            if use_qbias:
                t = consts.tile([mg, 1], F32, tag=f"qb{g}", name=f"qb{g}")
                nc.sync.dma_start(out=t, in_=d[f"qbias{g}"][:, :])
                qbias_t.append(t)
            if use_kbias:
                t = consts.tile([mg, 1], F32, tag=f"kb{g}", name=f"kb{g}")
                nc.sync.dma_start(out=t, in_=d[f"kbias{g}"][:, :])
                kbias_t.append(t)

    # ---- qproj: qpT_g[g] [32*hg, 150] fp16 = (q_w @ q.T + q_b) * NORM ----
    qpT = []
    for g, (hg, h0) in enumerate(HGROUPS):
        mg = 32 * hg
        moff = 32 * h0
        ps = psum.tile([mg, QS], F32, tag="ps", name=f"qproj_ps{g}")
        for kb in range(2):
            nc.tensor.matmul(
                ps[0:mg, 0:QS],
                qwT[kb][:, moff:moff + mg],
                qT[kb][:, 0:QS],
                start=(kb == 0),
                stop=(kb == 1),
            )
        t = persist.tile([mg, QS], F16, tag=f"qpT{g}", name=f"qpT{g}")
        bias = qbias_t[g][0:mg, 0:1] if use_qbias else 0.0
        nc.scalar.activation(
            t[0:mg, 0:QS], ps[0:mg, 0:QS],
            mybir.ActivationFunctionType.Identity,
            bias=bias, scale=NORM_FACT,
        )
        qpT.append(t)

    # ---- block-diagonal lhsT staging tiles (both rounds) ----
    # stg[(r, g, qb)]: [K_g, M_p] fp16, K_g = 32*hg + 1 (ones row last),
    # block r' at rows 32r'..32r'+32, cols r'*qs..(r'+1)*qs.
    stg = {}
    for r in range(2):
        for g, (hg, h0) in enumerate(HGROUPS):
            kg = 32 * hg + 1
            for qb, (q0, qs) in enumerate(QBLOCKS):
                mp = hg * qs
                t = persist.tile([kg, 126], F16, tag=f"stg_{r}_{g}_{qb}",
                                 name=f"stg_{r}_{g}_{qb}")
                nc.vector.memset(t, 0.0)
                qa = r * ROUND_Q + q0
                for rr in range(hg):
                    nc.vector.tensor_copy(
                        t[32 * rr:32 * rr + 32, rr * qs:(rr + 1) * qs],
                        qpT[g][32 * rr:32 * rr + 32, qa:qa + qs],
                    )
                nc.vector.memset(t[kg - 1:kg, 0:mp], 1.0)
                stg[(r, g, qb)] = t

    # ---- kproj: kp[g] [32*hg + 1, sp] fp16, bias row last ----
    kp = []
    for g, (hg, h0) in enumerate(HGROUPS):
        kg = 32 * hg + 1
        t = persist.tile([kg, sp], F16, tag=f"kp{g}", name=f"kp{g}")
        nc.sync.dma_start(out=t[kg - 1:kg, :], in_=d["maskb"][0:1, :])
        kp.append(t)

    # columns this core projects (all of sp, or its half under half_k)
    kw_cols = sp // 2 if half_k else sp
    if half_k:
        kph = dram.tile([256, kw_cols], F16, tag="kph", name="kph")
        kpg = dram.tile([512, kw_cols], F16, tag="kpg", name="kpg")
        kp_half = []
        for g, (hg, h0) in enumerate(HGROUPS):
            mg = 32 * hg
            t = persist.tile([mg, kw_cols], F16, tag=f"kph{g}",
                             name=f"kph{g}")
            kp_half.append(t)

    for c0, cw in _chunks(kw_cols, 2048):
        kin = []
        for kb in range(2):
            t = work.tile([128, 2048], F16, tag=f"kin{kb}", bufs=2,
                          name=f"kin{kb}_{c0}")
            nc.sync.dma_start(out=t[:, 0:cw],
                              in_=d["k"][kb * 128:(kb + 1) * 128, c0:c0 + cw])
            kin.append(t)
        for g, (hg, h0) in enumerate(HGROUPS):
            mg = 32 * hg
            moff = 32 * h0
            ps = psum.tile([mg, 2048], F32, tag="ps", name=f"kproj_ps{g}_{c0}")
            for js, nw in _chunks(cw, 512):
                for kb in range(2):
                    nc.tensor.matmul(
                        ps[0:mg, js:js + nw],
                        kwT[kb][:, moff:moff + mg],
                        kin[kb][:, js:js + nw],
                        start=(kb == 0),
                        stop=(kb == 1),
                    )
            bias = kbias_t[g][0:mg, 0:1] if use_kbias else 0.0
            dst = kp_half[g] if half_k else kp[g]
            nc.scalar.activation(
                dst[0:mg, c0:c0 + cw], ps[0:mg, 0:cw],
                mybir.ActivationFunctionType.Identity,
                bias=bias, scale=1.0,
            )

    if half_k:
        # bounce kp halves to DRAM, AllGather within (even, odd) pairs,
        # reload both halves into the full-width kp tiles
        for g, (hg, h0) in enumerate(HGROUPS):
            mg = 32 * hg
            moff = 32 * h0
            nc.gpsimd.dma_start(kph[moff:moff + mg, :], kp_half[g][0:mg, :])
        nc.gpsimd.collective_compute(
            "AllGather", mybir.AluOpType.bypass,
            replica_groups=[[0, 1], [2, 3], [4, 5], [6, 7]],
            ins=[kph.opt()], outs=[kpg.opt()],
        )
        for g, (hg, h0) in enumerate(HGROUPS):
            mg = 32 * hg
            moff = 32 * h0
            nc.gpsimd.dma_start(kp[g][0:mg, 0:kw_cols],
                                kpg[moff:moff + mg, :])
            nc.gpsimd.dma_start(kp[g][0:mg, kw_cols:sp],
                                kpg[256 + moff:256 + moff + mg, :])

    # ---- rounds: scores -> exp(+accum) -> sums -> normalize -> out ----
    for r in range(2):
        expb = {}
        sums = {}
        for qb, (q0, qs) in enumerate(QBLOCKS):
            t = small.tile([126, 3], F32, tag=f"sums_{r}_{qb}", bufs=1,
                           name=f"sums_{r}_{qb}")
            nc.vector.memset(t, 0.0)
            sums[qb] = t

        for qb, (q0, qs) in enumerate(QBLOCKS):
            for g, (hg, h0) in enumerate(HGROUPS):
                kg = 32 * hg + 1
                mp = hg * qs
                eb = work.tile([126, sp], F16, tag=f"expb_{g}_{qb}", bufs=1,
                               name=f"expb_{r}_{g}_{qb}")
                expb[(g, qb)] = eb
                parts = small.tile([126, n_parts], F32, tag="parts", bufs=3,
                                   name=f"parts_{r}_{g}_{qb}")
                lhs = stg[(r, g, qb)]
                for ci, (c0, cw) in enumerate(_chunks(sp, 2048)):
                    ps = psum.tile([126, 2048], F32, tag="ps",
                                   name=f"sc_ps_{r}_{g}_{qb}_{c0}")
                    for js, nw in _chunks(cw, 512):
                        nc.tensor.matmul(
                            ps[0:mp, js:js + nw],
                            lhs[0:kg, 0:mp],
                            kp[g][0:kg, c0 + js:c0 + js + nw],
                            start=True, stop=True,
                        )
                    nc.scalar.activation(
                        eb[0:mp, c0:c0 + cw], ps[0:mp, 0:cw],
                        mybir.ActivationFunctionType.Exp,
                        accum_out=parts[0:mp, ci:ci + 1],
                    )
                nc.vector.tensor_reduce(
                    sums[qb][0:mp, g:g + 1], parts[0:mp, 0:n_parts],
                    axis=mybir.AxisListType.X, op=mybir.AluOpType.add,
                )

        # per-query totals -> reciprocal -> broadcast to pass layout
        recP = {}
        for qb, (q0, qs) in enumerate(QBLOCKS):
            fps = psum.tile([qs, 3], F32, tag="ps", name=f"fold_ps_{r}_{qb}")
            nc.tensor.matmul(fps[0:qs, 0:3], foldc[qs][0:3 * qs, 0:qs],
                             sums[qb][0:3 * qs, 0:3], start=True, stop=True)
            tot = small.tile([qs, 1], F32, tag="tot", name=f"tot_{r}_{qb}")
            nc.vector.tensor_reduce(tot[0:qs, 0:1], fps[0:qs, 0:3],
                                    axis=mybir.AxisListType.X,
                                    op=mybir.AluOpType.add)
            rec = small.tile([qs, 1], F32, tag="rec", name=f"rec_{r}_{qb}")
            nc.vector.reciprocal(rec[0:qs, 0:1], tot[0:qs, 0:1])
            ups = psum.tile([3 * qs, 1], F32, tag="ps", name=f"unf_ps_{r}_{qb}")
            nc.tensor.matmul(ups[0:3 * qs, 0:1], unfoldc[qs][0:qs, 0:3 * qs],
                             rec[0:qs, 0:1], start=True, stop=True)
            rp = small.tile([126, 1], F32, tag=f"recP{qb}", bufs=2,
                            name=f"recP_{r}_{qb}")
            nc.vector.tensor_copy(rp[0:3 * qs, 0:1], ups[0:3 * qs, 0:1])
            recP[qb] = rp

        # normalize in place and write out
        out_r = d["out"][:].rearrange("q (h s) -> h q s", h=NH)
        for qb, (q0, qs) in enumerate(QBLOCKS):
            for g, (hg, h0) in enumerate(HGROUPS):
                mp = hg * qs
                eb = expb[(g, qb)]
                nc.vector.tensor_scalar_mul(
                    eb[0:mp, 0:sp], eb[0:mp, 0:sp], recP[qb][0:mp, 0:1],
                )
                qa = r * ROUND_Q + q0
                nc.sync.dma_start(
                    out=out_r[h0:h0 + hg, qa:qa + qs, :],
                    in_=eb[0:mp, 0:sp],
                )


_CACHED = {}


def _build(use_qbias, use_kbias, sp, half_k=False):
    key = (use_qbias, use_kbias, sp, half_k)
    if key in _CACHED:
        return _CACHED[key]
    nc = bacc.Bacc("TRN2", target_bir_lowering=False, debug=False)
    d = {}
    d["qT"] = nc.dram_tensor("qT", [256, QS], F16, kind="ExternalInput")
    d["k"] = nc.dram_tensor("k", [256, sp // 2 if half_k else sp], F16,
                            kind="ExternalInput")
    d["maskb"] = nc.dram_tensor("maskb", [1, sp], F16, kind="ExternalInput")
    d["q_wT"] = nc.dram_tensor("q_wT", [256, 256], F16, kind="ExternalInput")
    d["k_wT"] = nc.dram_tensor("k_wT", [256, 256], F16, kind="ExternalInput")
    for qs in (42, 33):
        d[f"fold{qs}"] = nc.dram_tensor(f"fold{qs}", [3 * qs, qs], F32,
                                        kind="ExternalInput")
        d[f"unfold{qs}"] = nc.dram_tensor(f"unfold{qs}", [qs, 3 * qs], F32,
                                          kind="ExternalInput")
    if use_qbias:
        for g, (hg, h0) in enumerate(HGROUPS):
            d[f"qbias{g}"] = nc.dram_tensor(f"qbias{g}", [32 * hg, 1], F32,
                                            kind="ExternalInput")
    if use_kbias:
        for g, (hg, h0) in enumerate(HGROUPS):
            d[f"kbias{g}"] = nc.dram_tensor(f"kbias{g}", [32 * hg, 1], F32,
                                            kind="ExternalInput")
    d["out"] = nc.dram_tensor("out", [QS, NH * sp], OUT_DTYPE,
                              kind="ExternalOutput")
    from contextlib import ExitStack
    with tile.TileContext(nc) as tc:
        with ExitStack() as ctx:
            _emit(nc, tc, ctx, d, use_qbias, use_kbias, sp, half_k=half_k)
    nc.compile()
    _CACHED[key] = nc
    return nc


# ---------------------------------------------------------------------------
# Host marshaling (jax CPU backend -- numpy fp16 conversions are ~40MB/s).
# ---------------------------------------------------------------------------

_CPU_FNS = {}


def _cpu_fn(name):
    if name in _CPU_FNS:
        return _CPU_FNS[name]

    def marshal_qw(q, q_w, k_w):
        # per-core qT: [4,300,256] -> [4,2,256,150] -> [2048,150] f16
        qT = q.reshape(B, 2, QS, QUERY_DIM).transpose(0, 1, 3, 2)
        qT = qT.reshape(NCORES * QUERY_DIM, QS).astype(jnp.float16)
        qwT = jnp.tile(q_w.T.astype(jnp.float16), (NCORES, 1))
        kwT = jnp.tile(k_w.T.astype(jnp.float16), (NCORES, 1))
        return qT, qwT, kwT

    def marshal_k_full(k, mask):
        # per-core k: [4,256,100,100] -> dup x2 -> [2048,10000] f8
        k8 = k.reshape(B, 1, QUERY_DIM, S).astype(jnp.float16)
        k8 = jnp.broadcast_to(k8, (B, 2, QUERY_DIM, S))
        k8 = k8.reshape(NCORES * QUERY_DIM, S)
        mb = jnp.where(mask.reshape(B, 1, S), jnp.float16(MASK_NEG),
                       jnp.float16(0.0))
        mb = jnp.broadcast_to(mb, (B, 2, S)).reshape(NCORES, S)
        return k8, mb

    def marshal_k_compact(k, idx):
        # gather kept columns: k [4,256,10000] f32, idx [4,SPC] int32.
        # column-half split: core (b,0) gets cols 0:SPC/2, (b,1) the rest
        # (recombined on-device by the paired kp AllGather)
        kc = jnp.take_along_axis(k.reshape(B, QUERY_DIM, S), idx[:, None, :],
                                 axis=2).astype(jnp.float16)
        kc = kc.reshape(B, QUERY_DIM, 2, SPC // 2).transpose(0, 2, 1, 3)
        return kc.reshape(NCORES * QUERY_DIM, SPC // 2)

    def convert_full(o16):
        # [1200, 80000] f16 -> [4,300,8,100,100] f32
        return o16.astype(jnp.float32).reshape(B, Q, NH, H, W)

    def convert_compact(o16, gidx):
        # o16 [1200, 8*SPC] f16, gidx [4, S] int32 (SPC = zero dummy)
        v = o16.reshape(B, Q, NH, SPC)
        v = jnp.concatenate([v, jnp.zeros((B, Q, NH, 1), jnp.float16)],
                            axis=3)
        full = jnp.take_along_axis(v, gidx[:, None, None, :], axis=3)
        return full.astype(jnp.float32).reshape(B, Q, NH, H, W)

    def convert_compact_shard(o16, gidx):
        # o16 [QS, 8*SPC] f16 (one core), gidx [S] int32
        v = o16.reshape(QS, NH, SPC)
        v = jnp.concatenate([v, jnp.zeros((QS, NH, 1), jnp.float16)], axis=2)
        full = jnp.take_along_axis(v, gidx[None, None, :], axis=2)
        return full.astype(jnp.float32).reshape(QS, NH, H, W)

    def convert_full_shard(o16):
        return o16.astype(jnp.float32).reshape(QS, NH, H, W)

    fns = {"marshal_qw": marshal_qw, "marshal_k_full": marshal_k_full,
           "marshal_k_compact": marshal_k_compact,
           "convert_full": convert_full, "convert_compact": convert_compact,
           "convert_compact_shard": convert_compact_shard,
           "convert_full_shard": convert_full_shard}
    for n, f in fns.items():
        _CPU_FNS[n] = jax.jit(f, backend="cpu")
    return _CPU_FNS[name]


def _prepare(q, k, mask, q_w, q_b, k_w, k_b):
    """Marshal inputs. Returns (nc, global_input_dict, postprocess).

    Global arrays stack the 8 per-core shards on axis 0 (core order =
    (batch, query-half) lexicographic), matching shard_map's P("core")."""
    use_qbias = bool(np.any(q_b != 0))
    use_kbias = bool(np.any(k_b != 0))

    mask = np.asarray(mask).reshape(B, S)
    counts = (~mask).sum(axis=1)
    compact = counts.max() <= SPC

    qT, qwT, kwT = (np.asarray(a) for a in
                    _cpu_fn("marshal_qw")(q, q_w, k_w))
    g = {"qT": qT, "q_wT": qwT, "k_wT": kwT}

    if compact:
        idx = np.zeros((B, SPC), np.int32)
        gidx = np.full((B, S), SPC, np.int32)
        maskb = np.full((B, SPC), np.float16(MASK_NEG))
        for b in range(B):
            kept = np.nonzero(~mask[b])[0]
            n = len(kept)
            idx[b, :n] = kept
            idx[b, n:] = kept[-1] if n else 0
            gidx[b, kept] = np.arange(n, dtype=np.int32)
            maskb[b, :n] = np.float16(0.0)
        g["k"] = np.asarray(_cpu_fn("marshal_k_compact")(
            k.reshape(B, QUERY_DIM, S), idx))
        g["maskb"] = np.broadcast_to(
            maskb[:, None], (B, 2, SPC)).reshape(NCORES, SPC).copy()
        sp = SPC
        conv = _cpu_fn("convert_compact")
        conv_shard = _cpu_fn("convert_compact_shard")

        def post(o16):
            return np.asarray(conv(o16, gidx))

        def post_shard(c, part):
            return np.asarray(conv_shard(part, gidx[c // 2]))
    else:
        k16, mb = _cpu_fn("marshal_k_full")(k, mask.reshape(B, H, W))
        g["k"] = np.asarray(k16)
        g["maskb"] = np.asarray(mb)
        sp = S
        conv = _cpu_fn("convert_full")
        conv_shard = _cpu_fn("convert_full_shard")

        def post(o16):
            return np.asarray(conv(o16))

        def post_shard(c, part):
            return np.asarray(conv_shard(part))

    for fname, arr in _fold_consts().items():
        g[fname] = np.tile(arr, (NCORES, 1))
    if use_qbias:
        qb_scaled = (q_b.astype(np.float32) * NORM_FACT).reshape(256, 1)
        for gi, (hg, h0) in enumerate(HGROUPS):
            g[f"qbias{gi}"] = np.tile(
                np.ascontiguousarray(qb_scaled[32 * h0:32 * h0 + 32 * hg]),
                (NCORES, 1))
    if use_kbias:
        kb_col = k_b.astype(np.float32).reshape(256, 1)
        for gi, (hg, h0) in enumerate(HGROUPS):
            g[f"kbias{gi}"] = np.tile(
                np.ascontiguousarray(kb_col[32 * h0:32 * h0 + 32 * hg]),
                (NCORES, 1))

    nc = _build(use_qbias, use_kbias, sp, half_k=compact)
    return nc, g, post, post_shard


def make_in_maps(q, k, mask, q_w, q_b, k_w, k_b):
    """Per-core input dicts + postprocess (sim / native-path use)."""
    nc, g, post, _ = _prepare(q, k, mask, q_w, q_b, k_w, k_b)
    in_maps = []
    for c in range(NCORES):
        m = {}
        for name, arr in g.items():
            rows = arr.shape[0] // NCORES
            m[name] = np.ascontiguousarray(arr[c * rows:(c + 1) * rows])
        in_maps.append(m)
    return nc, in_maps, post


# ---------------------------------------------------------------------------
# Execution: cached jit over the bass_exec custom call (axon PJRT path).
# ---------------------------------------------------------------------------

_RUNNERS = {}


def _get_runner(nc):
    key = id(nc)
    if key in _RUNNERS:
        return _RUNNERS[key]

    from concourse.bass2jax import (_bass_exec_p, install_neuronx_cc_hook,
                                    partition_id_tensor)
    from jax.sharding import Mesh, PartitionSpec, NamedSharding
    try:
        from jax.experimental.shard_map import shard_map
    except ImportError:
        shard_map = jax.shard_map

    install_neuronx_cc_hook()

    partition_name = (nc.partition_id_tensor.name
                      if nc.partition_id_tensor else None)
    in_names, out_names, out_avals = [], [], []
    for alloc in nc.m.functions[0].allocations:
        if not isinstance(alloc, mybir.MemoryLocationSet):
            continue
        name = alloc.memorylocations[0].name
        if alloc.kind == "ExternalInput":
            if name != partition_name:
                in_names.append(name)
        elif alloc.kind == "ExternalOutput":
            out_names.append(name)
            out_avals.append(jax.core.ShapedArray(
                tuple(alloc.tensor_shape), mybir.dt.np(alloc.dtype)))
    n_params = len(in_names)
    bind_names = tuple(in_names + ([partition_name] if partition_name else []))

    devices = jax.devices()[:NCORES]
    mesh = Mesh(np.asarray(devices), ("core",))
    P = PartitionSpec

    def _body(*args):
        operands = list(args)
        if partition_name:
            operands.append(partition_id_tensor())
        outs = _bass_exec_p.bind(
            *operands, out_avals=tuple(out_avals), in_names=bind_names,
            out_names=tuple(out_names), lowering_input_output_aliases=(),
            sim_require_finite=True, sim_require_nnan=True, nc=nc)
        return tuple(outs)

    sh = NamedSharding(mesh, P("core"))

    def _make_jit():
        return jax.jit(shard_map(
            _body, mesh=mesh, in_specs=(P("core"),) * n_params,
            out_specs=(P("core"),) * len(out_names), check_rep=False))

    sharded = None
    try:
        # AOT-compile with the bass effect suppressed: C++ fast-path
        # dispatch (the effectful path re-enters Python every call).
        from concourse.bass2jax import fast_dispatch_compile
        arg_structs = []
        for name in in_names:
            alloc_shape = None
            for alloc in nc.m.functions[0].allocations:
                if (isinstance(alloc, mybir.MemoryLocationSet)
                        and alloc.memorylocations[0].name == name):
                    alloc_shape = (NCORES * alloc.tensor_shape[0],
                                   *alloc.tensor_shape[1:])
                    dt = mybir.dt.np(alloc.dtype)
                    break
            arg_structs.append(
                jax.ShapeDtypeStruct(alloc_shape, dt, sharding=sh))
        sharded = fast_dispatch_compile(
            lambda: _make_jit().lower(*arg_structs).compile())
    except Exception:
        sharded = None
    if sharded is None:
        sharded = _make_jit()
    runner = {"fn": sharded, "in_names": in_names, "out_names": out_names,
              "sharding": sh, "const_cache": {}}
    _RUNNERS[key] = runner
    return runner


def _run_axon(nc, global_inputs, post_shard):
    """Dispatch via the cached jit; fetch + convert shards pipelined
    (converts hide inside the serialized tunnel transfers)."""
    from concurrent.futures import ThreadPoolExecutor
    r = _get_runner(nc)
    # device-cache the fold/unfold constants (identical every call)
    args = []
    for name in r["in_names"]:
        arr = global_inputs[name]
        if name.startswith(("fold", "unfold")):
            darr = r["const_cache"].get(name)
            if darr is None:
                darr = jax.device_put(np.asarray(arr), r["sharding"])
                r["const_cache"][name] = darr
            args.append(darr)
        else:
            args.append(arr)
    out = r["fn"](*args)[0]
    shards = sorted(out.addressable_shards,
                    key=lambda s: s.index[0].start or 0)
    final = np.empty((B, Q, NH, H, W), np.float32)

    def work(c):
        part = np.asarray(shards[c].data)
        b, qh = c // 2, c % 2
        final[b, qh * QS:(qh + 1) * QS] = post_shard(c, part)

    with ThreadPoolExecutor(4) as ex:
        list(ex.map(work, range(NCORES)))
    return final


def kernel(q, k, mask, q_w, q_b, k_w, k_b):
    import os
    nc, g, post, post_shard = _prepare(q, k, mask, q_w, q_b, k_w, k_b)
    if axon_active() and not os.environ.get("BASS_TRACE"):
        return _run_axon(nc, g, post_shard)
    from concourse.bass_utils import run_bass_kernel_spmd
    in_maps = []
    for c in range(NCORES):
        m = {}
        for name, arr in g.items():
            rows = arr.shape[0] // NCORES
            m[name] = np.ascontiguousarray(arr[c * rows:(c + 1) * rows])
        in_maps.append(m)
    res = run_bass_kernel_spmd(nc, in_maps, core_ids=list(range(NCORES)))
    out16 = np.concatenate([r["out"] for r in res.results], axis=0)
    return post(out16)


# revision 29
# speedup vs baseline: 1.1648x; 1.0567x over previous
# Trainium2 Bass kernel for nn_MHAttentionMap (DETR-style attention map).
#
# Reference computation:
#   qp = q @ q_w.T + q_b                       [b, Q, 256]
#   kp = 1x1conv(k, k_w) + k_b                 [b, 256, H, W]
#   scores[b,q,n,s] = (qh*NORM) . kh           [b, Q, 8, H*W]
#   scores[mask] = -inf ; softmax over flattened (n, H, W) per (b, q)
#
# Sharding: 8 cores = (batch 0..3) x (query half 0..1); 150 queries/core.
# The softmax axis (heads x spatial) lives entirely on one core. The only
# cross-core exchange is a paired AllGather of the fp16 kproj halves:
# both cores of a batch pair need the same kp, so each uploads + projects
# half of the (compacted) k columns and gathers the other half on-device
# (input IO -44% vs duplicating k, output identical bit-for-bit).
#
# Masked-column compaction: masked (h,w) positions are exactly 0 in the
# output (exp(-inf)) and contribute nothing to the softmax sum, and the
# mask is known on the host per batch. The host gathers only the kept
# k-columns (padded to a static SPC with -30000-bias pad slots), the
# device computes scores/exp on SPC ~= S/2 columns, and the host
# scatters the compact output back to the full [.., 100, 100] layout via
# an XLA gather (masked positions read a zero column). This halves the
# dominant host<->device IO (k upload + attention-map download) and the
# device compute. If a mask ever keeps more than SPC columns, we fall
# back to the full-width program.
#
# Per-core device program (identical on all cores, different data):
#   - all projection inputs arrive as fp16 (halves H2D, 1-pass PE matmuls)
#   - qproj on PE -> qpT group tiles (fp16), NORM_FACT folded in
#   - kproj on PE (K=256) -> kp tiles [97/97/65, sp] fp16 grouped as
#     heads (0-2), (3-5), (6-7); last row of each = bias row
#     (0 / -30000, fp16, marshaled on host)
#   - scores: block-diagonal matmuls packing (heads-in-group x query-block)
#     into M<=126 with a ones-row in lhsT so the mask bias adds inside the
#     matmul (K = 32*hg + 1)
#   - exp on ACT directly from PSUM into per-pass fp16 buffers with
#     accum_out partial row sums; fold/unfold 0/1 matmuls (host constants)
#     reduce per-(head,query) sums into per-query totals and broadcast the
#     reciprocal back to the pass layout
#   - in-place DVE normalize, DMA out
#
# Host side: marshaling and the f16->f32 output conversion run through
# jax's CPU backend (multithreaded XLA) -- numpy's fp16 paths are ~40MB/s.
# Under axon the dispatch goes through a module-cached jax.jit of the
# bass_exec custom call (the stock run_bass_kernel_spmd rebuilds the jit
# and re-uploads 192MB of zero output buffers every call).

import numpy as np

import jax
import jax.numpy as jnp

import concourse.bacc as bacc
import concourse.bass as bass
import concourse.mybir as mybir
import concourse.tile as tile

try:
    from concourse._compat import axon_active
except ImportError:
    import os as _os

    def axon_active():
        return (bool(_os.environ.get("AXON_TERMINAL_JOB_NAME"))
                or _os.environ.get("AXON_H4_ENABLED") == "1")

QUERY_DIM = 256
HIDDEN = 256
NH = 8
HD = HIDDEN // NH  # 32
NORM_FACT = float(HIDDEN / NH) ** (-0.5)

B = 4
Q = 300
H = 100
W = 100
S = H * W  # 10000
NCORES = 8
QS = Q // 2  # 150 queries per core

# compact spatial width: P(Binomial(10000,1/2) > 5248) ~ 4e-7 per batch
SPC = 5248

# head groups: (#heads, first head)
HGROUPS = [(3, 0), (3, 3), (2, 6)]
# query rounds of 75, each split into blocks of (42, 33)
ROUND_Q = 75
QBLOCKS = [(0, 42), (42, 33)]

MASK_NEG = -30000.0

# fp16 output halves the dominant HBM write; verified <2e-3 rel err.
OUT_DTYPE = mybir.dt.float16

F32 = mybir.dt.float32
F16 = mybir.dt.float16
# NOTE: an fp8(E3M4) kproj datapath (k + k_w quantized, weights pre-scaled
# x32 out of the subnormal zone) was tried and REJECTED: CoreSim scale-rel
# error 2.6e-2 vs the 2e-2 gate (f16 path: 7.3e-4). Keep k/k_w in f16.


def _chunks(total, size):
    out = []
    off = 0
    while off < total:
        out.append((off, min(size, total - off)))
        off += size
    return out


def _fold_consts():
    # fold[qs*r + j, j] = 1 folds 3 stacked per-head rows into per-query;
    # unfold is its transpose (broadcast back to pass layout).
    consts = {}
    for qs in (42, 33):
        fold = np.zeros((3 * qs, qs), np.float32)
        for r in range(3):
            fold[qs * r + np.arange(qs), np.arange(qs)] = 1.0
        consts[f"fold{qs}"] = fold
        consts[f"unfold{qs}"] = np.ascontiguousarray(fold.T)
    return consts


def _emit(nc, tc, ctx, d, use_qbias, use_kbias, sp, half_k=False):
    """Emit the per-core program. d: dict of DRAM tensor handles.

    half_k: each core of a (batch) pair uploads and projects only half of
    the k columns; the fp16 kp halves are exchanged with a paired
    AllGather through DRAM bounce tiles (input IO -44%)."""
    consts = ctx.enter_context(tc.tile_pool(name="consts", bufs=1))
    persist = ctx.enter_context(tc.tile_pool(name="persist", bufs=1))
    work = ctx.enter_context(tc.tile_pool(name="work", bufs=3))
    small = ctx.enter_context(tc.tile_pool(name="small", bufs=4))
    psum = ctx.enter_context(tc.tile_pool(name="psum", bufs=2, space="PSUM"))
    if half_k:
        dram = ctx.enter_context(tc.tile_pool(name="dram", bufs=1,
                                              space="DRAM"))

    n_parts = len(_chunks(sp, 2048))

    # ---- load constants ----
    qwT = []
    kwT = []
    for kb in range(2):
        t = consts.tile([128, 256], F16, tag=f"qwT{kb}", name=f"qwT{kb}")
        nc.sync.dma_start(out=t, in_=d["q_wT"][kb * 128:(kb + 1) * 128, :])
        qwT.append(t)
        t2 = consts.tile([128, 256], F16, tag=f"kwT{kb}", name=f"kwT{kb}")
        nc.sync.dma_start(out=t2, in_=d["k_wT"][kb * 128:(kb + 1) * 128, :])
        kwT.append(t2)
    qT = []
    for kb in range(2):
        t = consts.tile([128, QS], F16, tag=f"qT{kb}", name=f"qT{kb}")
        nc.sync.dma_start(out=t, in_=d["qT"][kb * 128:(kb + 1) * 128, :])
        qT.append(t)
    foldc = {}
    unfoldc = {}
    for qs in (42, 33):
        f = consts.tile([3 * qs, qs], F32, tag=f"fold{qs}", name=f"fold{qs}")
        nc.sync.dma_start(out=f, in_=d[f"fold{qs}"][:, :])
        foldc[qs] = f
        u = consts.tile([qs, 3 * qs], F32, tag=f"unfold{qs}", name=f"unfold{qs}")
        nc.sync.dma_start(out=u, in_=d[f"unfold{qs}"][:, :])
        unfoldc[qs] = u
    qbias_t = []
    kbias_t = []
    if use_qbias or use_kbias:
        for g, (hg, h0) in enumerate(HGROUPS):
            mg = 32 * hg
            if use_qbias:
                t = consts.tile([mg, 1], F32, tag=f"qb{g}", name=f"qb{g}")
                nc.sync.dma_start(out=t, in_=d[f"qbias{g}"][:, :])
                qbias_t.append(t)
            if use_kbias:
                t = consts.tile([mg, 1], F32, tag=f"kb{g}", name=f"kb{g}")
                nc.sync.dma_start(out=t, in_=d[f"kbias{g}"][:, :])
                kbias_t.append(t)

    # ---- qproj: qpT_g[g] [32*hg, 150] fp16 = (q_w @ q.T + q_b) * NORM ----
    qpT = []
    for g, (hg, h0) in enumerate(HGROUPS):
        mg = 32 * hg
        moff = 32 * h0
        ps = psum.tile([mg, QS], F32, tag="ps", name=f"qproj_ps{g}")
        for kb in range(2):
            nc.tensor.matmul(
                ps[0:mg, 0:QS],
                qwT[kb][:, moff:moff + mg],
                qT[kb][:, 0:QS],
                start=(kb == 0),
                stop=(kb == 1),
            )
        t = persist.tile([mg, QS], F16, tag=f"qpT{g}", name=f"qpT{g}")
        bias = qbias_t[g][0:mg, 0:1] if use_qbias else 0.0
        nc.scalar.activation(
            t[0:mg, 0:QS], ps[0:mg, 0:QS],
            mybir.ActivationFunctionType.Identity,
            bias=bias, scale=NORM_FACT,
        )
        qpT.append(t)

    # ---- block-diagonal lhsT staging tiles (both rounds) ----
    # stg[(r, g, qb)]: [K_g, M_p] fp16, K_g = 32*hg + 1 (ones row last),
    # block r' at rows 32r'..32r'+32, cols r'*qs..(r'+1)*qs.
    stg = {}
    for r in range(2):
        for g, (hg, h0) in enumerate(HGROUPS):
            kg = 32 * hg + 1
            for qb, (q0, qs) in enumerate(QBLOCKS):
                mp = hg * qs
                t = persist.tile([kg, 126], F16, tag=f"stg_{r}_{g}_{qb}",
                                 name=f"stg_{r}_{g}_{qb}")
                nc.vector.memset(t, 0.0)
                qa = r * ROUND_Q + q0
                for rr in range(hg):
                    nc.vector.tensor_copy(
                        t[32 * rr:32 * rr + 32, rr * qs:(rr + 1) * qs],
                        qpT[g][32 * rr:32 * rr + 32, qa:qa + qs],
                    )
                nc.vector.memset(t[kg - 1:kg, 0:mp], 1.0)
                stg[(r, g, qb)] = t

    # ---- kproj: kp[g] [32*hg + 1, sp] fp16, bias row last ----
    kp = []
    for g, (hg, h0) in enumerate(HGROUPS):
        kg = 32 * hg + 1
        t = persist.tile([kg, sp], F16, tag=f"kp{g}", name=f"kp{g}")
        nc.sync.dma_start(out=t[kg - 1:kg, :], in_=d["maskb"][0:1, :])
        kp.append(t)

    # columns this core projects (all of sp, or its half under half_k)
    kw_cols = sp // 2 if half_k else sp
    if half_k:
        kph = dram.tile([256, kw_cols], F16, tag="kph", name="kph")
        kpg = dram.tile([512, kw_cols], F16, tag="kpg", name="kpg")
        kp_half = []
        for g, (hg, h0) in enumerate(HGROUPS):
            mg = 32 * hg
            t = persist.tile([mg, kw_cols], F16, tag=f"kph{g}",
                             name=f"kph{g}")
            kp_half.append(t)

    for c0, cw in _chunks(kw_cols, 2048):
        kin = []
        for kb in range(2):
            t = work.tile([128, 2048], F16, tag=f"kin{kb}", bufs=2,
                          name=f"kin{kb}_{c0}")
            nc.sync.dma_start(out=t[:, 0:cw],
                              in_=d["k"][kb * 128:(kb + 1) * 128, c0:c0 + cw])
            kin.append(t)
        for g, (hg, h0) in enumerate(HGROUPS):
            mg = 32 * hg
            moff = 32 * h0
            ps = psum.tile([mg, 2048], F32, tag="ps", name=f"kproj_ps{g}_{c0}")
            for js, nw in _chunks(cw, 512):
                for kb in range(2):
                    nc.tensor.matmul(
                        ps[0:mg, js:js + nw],
                        kwT[kb][:, moff:moff + mg],
                        kin[kb][:, js:js + nw],
                        start=(kb == 0),
                        stop=(kb == 1),
                    )
            bias = kbias_t[g][0:mg, 0:1] if use_kbias else 0.0
            dst = kp_half[g] if half_k else kp[g]
            nc.scalar.activation(
                dst[0:mg, c0:c0 + cw], ps[0:mg, 0:cw],
                mybir.ActivationFunctionType.Identity,
                bias=bias, scale=1.0,
            )

    if half_k:
        # bounce kp halves to DRAM, AllGather within (even, odd) pairs,
        # reload both halves into the full-width kp tiles
        for g, (hg, h0) in enumerate(HGROUPS):
            mg = 32 * hg
            moff = 32 * h0
            nc.gpsimd.dma_start(kph[moff:moff + mg, :], kp_half[g][0:mg, :])
        nc.gpsimd.collective_compute(
            "AllGather", mybir.AluOpType.bypass,
            replica_groups=[[0, 1], [2, 3], [4, 5], [6, 7]],
            ins=[kph.opt()], outs=[kpg.opt()],
        )
        for g, (hg, h0) in enumerate(HGROUPS):
            mg = 32 * hg
            moff = 32 * h0
            nc.gpsimd.dma_start(kp[g][0:mg, 0:kw_cols],
                                kpg[moff:moff + mg, :])
            nc.gpsimd.dma_start(kp[g][0:mg, kw_cols:sp],
                                kpg[256 + moff:256 + moff + mg, :])

    # ---- rounds: scores -> exp(+accum) -> sums -> normalize -> out ----
    for r in range(2):
        expb = {}
        sums = {}
        for qb, (q0, qs) in enumerate(QBLOCKS):
            t = small.tile([126, 3], F32, tag=f"sums_{r}_{qb}", bufs=1,
                           name=f"sums_{r}_{qb}")
            nc.vector.memset(t, 0.0)
            sums[qb] = t

        for qb, (q0, qs) in enumerate(QBLOCKS):
            for g, (hg, h0) in enumerate(HGROUPS):
                kg = 32 * hg + 1
                mp = hg * qs
                eb = work.tile([126, sp], F16, tag=f"expb_{g}_{qb}", bufs=1,
                               name=f"expb_{r}_{g}_{qb}")
                expb[(g, qb)] = eb
                parts = small.tile([126, n_parts], F32, tag="parts", bufs=3,
                                   name=f"parts_{r}_{g}_{qb}")
                lhs = stg[(r, g, qb)]
                for ci, (c0, cw) in enumerate(_chunks(sp, 2048)):
                    ps = psum.tile([126, 2048], F32, tag="ps",
                                   name=f"sc_ps_{r}_{g}_{qb}_{c0}")
                    for js, nw in _chunks(cw, 512):
                        nc.tensor.matmul(
                            ps[0:mp, js:js + nw],
                            lhs[0:kg, 0:mp],
                            kp[g][0:kg, c0 + js:c0 + js + nw],
                            start=True, stop=True,
                        )
                    nc.scalar.activation(
                        eb[0:mp, c0:c0 + cw], ps[0:mp, 0:cw],
                        mybir.ActivationFunctionType.Exp,
                        accum_out=parts[0:mp, ci:ci + 1],
                    )
                nc.vector.tensor_reduce(
                    sums[qb][0:mp, g:g + 1], parts[0:mp, 0:n_parts],
                    axis=mybir.AxisListType.X, op=mybir.AluOpType.add,
                )

        # per-query totals -> reciprocal -> broadcast to pass layout
        recP = {}
        for qb, (q0, qs) in enumerate(QBLOCKS):
            fps = psum.tile([qs, 3], F32, tag="ps", name=f"fold_ps_{r}_{qb}")
            nc.tensor.matmul(fps[0:qs, 0:3], foldc[qs][0:3 * qs, 0:qs],
                             sums[qb][0:3 * qs, 0:3], start=True, stop=True)
            tot = small.tile([qs, 1], F32, tag="tot", name=f"tot_{r}_{qb}")
            nc.vector.tensor_reduce(tot[0:qs, 0:1], fps[0:qs, 0:3],
                                    axis=mybir.AxisListType.X,
                                    op=mybir.AluOpType.add)
            rec = small.tile([qs, 1], F32, tag="rec", name=f"rec_{r}_{qb}")
            nc.vector.reciprocal(rec[0:qs, 0:1], tot[0:qs, 0:1])
            ups = psum.tile([3 * qs, 1], F32, tag="ps", name=f"unf_ps_{r}_{qb}")
            nc.tensor.matmul(ups[0:3 * qs, 0:1], unfoldc[qs][0:qs, 0:3 * qs],
                             rec[0:qs, 0:1], start=True, stop=True)
            rp = small.tile([126, 1], F32, tag=f"recP{qb}", bufs=2,
                            name=f"recP_{r}_{qb}")
            nc.vector.tensor_copy(rp[0:3 * qs, 0:1], ups[0:3 * qs, 0:1])
            recP[qb] = rp

        # normalize in place and write out
        out_r = d["out"][:].rearrange("q (h s) -> h q s", h=NH)
        for qb, (q0, qs) in enumerate(QBLOCKS):
            for g, (hg, h0) in enumerate(HGROUPS):
                mp = hg * qs
                eb = expb[(g, qb)]
                nc.vector.tensor_scalar_mul(
                    eb[0:mp, 0:sp], eb[0:mp, 0:sp], recP[qb][0:mp, 0:1],
                )
                qa = r * ROUND_Q + q0
                nc.sync.dma_start(
                    out=out_r[h0:h0 + hg, qa:qa + qs, :],
                    in_=eb[0:mp, 0:sp],
                )


_CACHED = {}


def _build(use_qbias, use_kbias, sp, half_k=False):
    key = (use_qbias, use_kbias, sp, half_k)
    if key in _CACHED:
        return _CACHED[key]
    nc = bacc.Bacc("TRN2", target_bir_lowering=False, debug=False)
    d = {}
    d["qT"] = nc.dram_tensor("qT", [256, QS], F16, kind="ExternalInput")
    d["k"] = nc.dram_tensor("k", [256, sp // 2 if half_k else sp], F16,
                            kind="ExternalInput")
    d["maskb"] = nc.dram_tensor("maskb", [1, sp], F16, kind="ExternalInput")
    d["q_wT"] = nc.dram_tensor("q_wT", [256, 256], F16, kind="ExternalInput")
    d["k_wT"] = nc.dram_tensor("k_wT", [256, 256], F16, kind="ExternalInput")
    for qs in (42, 33):
        d[f"fold{qs}"] = nc.dram_tensor(f"fold{qs}", [3 * qs, qs], F32,
                                        kind="ExternalInput")
        d[f"unfold{qs}"] = nc.dram_tensor(f"unfold{qs}", [qs, 3 * qs], F32,
                                          kind="ExternalInput")
    if use_qbias:
        for g, (hg, h0) in enumerate(HGROUPS):
            d[f"qbias{g}"] = nc.dram_tensor(f"qbias{g}", [32 * hg, 1], F32,
                                            kind="ExternalInput")
    if use_kbias:
        for g, (hg, h0) in enumerate(HGROUPS):
            d[f"kbias{g}"] = nc.dram_tensor(f"kbias{g}", [32 * hg, 1], F32,
                                            kind="ExternalInput")
    d["out"] = nc.dram_tensor("out", [QS, NH * sp], OUT_DTYPE,
                              kind="ExternalOutput")
    from contextlib import ExitStack
    with tile.TileContext(nc) as tc:
        with ExitStack() as ctx:
            _emit(nc, tc, ctx, d, use_qbias, use_kbias, sp, half_k=half_k)
    nc.compile()
    _CACHED[key] = nc
    return nc


# ---------------------------------------------------------------------------
# Host marshaling (jax CPU backend -- numpy fp16 conversions are ~40MB/s).
# ---------------------------------------------------------------------------

_CPU_FNS = {}


def _cpu_fn(name):
    if name in _CPU_FNS:
        return _CPU_FNS[name]

    def marshal_qw(q, q_w, k_w):
        # per-core qT: [4,300,256] -> [4,2,256,150] -> [2048,150] f16
        qT = q.reshape(B, 2, QS, QUERY_DIM).transpose(0, 1, 3, 2)
        qT = qT.reshape(NCORES * QUERY_DIM, QS).astype(jnp.float16)
        qwT = jnp.tile(q_w.T.astype(jnp.float16), (NCORES, 1))
        kwT = jnp.tile(k_w.T.astype(jnp.float16), (NCORES, 1))
        return qT, qwT, kwT

    def marshal_k_full(k, mask):
        # per-core k: [4,256,100,100] -> dup x2 -> [2048,10000] f8
        k8 = k.reshape(B, 1, QUERY_DIM, S).astype(jnp.float16)
        k8 = jnp.broadcast_to(k8, (B, 2, QUERY_DIM, S))
        k8 = k8.reshape(NCORES * QUERY_DIM, S)
        mb = jnp.where(mask.reshape(B, 1, S), jnp.float16(MASK_NEG),
                       jnp.float16(0.0))
        mb = jnp.broadcast_to(mb, (B, 2, S)).reshape(NCORES, S)
        return k8, mb

    def marshal_k_compact(k, idx):
        # gather kept columns: k [4,256,10000] f32, idx [4,SPC] int32.
        # column-half split: core (b,0) gets cols 0:SPC/2, (b,1) the rest
        # (recombined on-device by the paired kp AllGather)
        kc = jnp.take_along_axis(k.reshape(B, QUERY_DIM, S), idx[:, None, :],
                                 axis=2).astype(jnp.float16)
        kc = kc.reshape(B, QUERY_DIM, 2, SPC // 2).transpose(0, 2, 1, 3)
        return kc.reshape(NCORES * QUERY_DIM, SPC // 2)

    def convert_full(o16):
        # [1200, 80000] f16 -> [4,300,8,100,100] f32
        return o16.astype(jnp.float32).reshape(B, Q, NH, H, W)

    def convert_compact(o16, gidx):
        # o16 [1200, 8*SPC] f16, gidx [4, S] int32 (SPC = zero dummy)
        v = o16.reshape(B, Q, NH, SPC)
        v = jnp.concatenate([v, jnp.zeros((B, Q, NH, 1), jnp.float16)],
                            axis=3)
        full = jnp.take_along_axis(v, gidx[:, None, None, :], axis=3)
        return full.astype(jnp.float32).reshape(B, Q, NH, H, W)

    def convert_compact_shard(o16, gidx):
        # o16 [QS, 8*SPC] f16 (one core), gidx [S] int32
        v = o16.reshape(QS, NH, SPC)
        v = jnp.concatenate([v, jnp.zeros((QS, NH, 1), jnp.float16)], axis=2)
        full = jnp.take_along_axis(v, gidx[None, None, :], axis=2)
        return full.astype(jnp.float32).reshape(QS, NH, H, W)

    def convert_full_shard(o16):
        return o16.astype(jnp.float32).reshape(QS, NH, H, W)

    fns = {"marshal_qw": marshal_qw, "marshal_k_full": marshal_k_full,
           "marshal_k_compact": marshal_k_compact,
           "convert_full": convert_full, "convert_compact": convert_compact,
           "convert_compact_shard": convert_compact_shard,
           "convert_full_shard": convert_full_shard}
    for n, f in fns.items():
        _CPU_FNS[n] = jax.jit(f, backend="cpu")
    return _CPU_FNS[name]


_FOLD_TILED = None


def _fold_consts_tiled():
    global _FOLD_TILED
    if _FOLD_TILED is None:
        _FOLD_TILED = {name: np.tile(arr, (NCORES, 1))
                       for name, arr in _fold_consts().items()}
    return _FOLD_TILED


def _prepare(q, k, mask, q_w, q_b, k_w, k_b):
    """Marshal inputs. Returns (nc, global_input_dict, postprocess).

    Global arrays stack the 8 per-core shards on axis 0 (core order =
    (batch, query-half) lexicographic), matching shard_map's P("core")."""
    use_qbias = bool(np.any(q_b != 0))
    use_kbias = bool(np.any(k_b != 0))

    mask = np.asarray(mask).reshape(B, S)
    counts = (~mask).sum(axis=1)
    compact = counts.max() <= SPC

    qT, qwT, kwT = (np.asarray(a) for a in
                    _cpu_fn("marshal_qw")(q, q_w, k_w))
    g = {"qT": qT, "q_wT": qwT, "k_wT": kwT}

    if compact:
        idx = np.zeros((B, SPC), np.int32)
        gidx = np.full((B, S), SPC, np.int32)
        maskb = np.full((B, SPC), np.float16(MASK_NEG))
        for b in range(B):
            kept = np.nonzero(~mask[b])[0]
            n = len(kept)
            idx[b, :n] = kept
            idx[b, n:] = kept[-1] if n else 0
            gidx[b, kept] = np.arange(n, dtype=np.int32)
            maskb[b, :n] = np.float16(0.0)
        g["k"] = np.asarray(_cpu_fn("marshal_k_compact")(
            k.reshape(B, QUERY_DIM, S), idx))
        g["maskb"] = np.broadcast_to(
            maskb[:, None], (B, 2, SPC)).reshape(NCORES, SPC).copy()
        sp = SPC
        conv = _cpu_fn("convert_compact")
        conv_shard = _cpu_fn("convert_compact_shard")

        def post(o16):
            return np.asarray(conv(o16, gidx))

        def post_shard(c, part):
            return np.asarray(conv_shard(part, gidx[c // 2]))
    else:
        k16, mb = _cpu_fn("marshal_k_full")(k, mask.reshape(B, H, W))
        g["k"] = np.asarray(k16)
        g["maskb"] = np.asarray(mb)
        sp = S
        conv = _cpu_fn("convert_full")
        conv_shard = _cpu_fn("convert_full_shard")

        def post(o16):
            return np.asarray(conv(o16))

        def post_shard(c, part):
            return np.asarray(conv_shard(part))

    g.update(_fold_consts_tiled())
    if use_qbias:
        qb_scaled = (q_b.astype(np.float32) * NORM_FACT).reshape(256, 1)
        for gi, (hg, h0) in enumerate(HGROUPS):
            g[f"qbias{gi}"] = np.tile(
                np.ascontiguousarray(qb_scaled[32 * h0:32 * h0 + 32 * hg]),
                (NCORES, 1))
    if use_kbias:
        kb_col = k_b.astype(np.float32).reshape(256, 1)
        for gi, (hg, h0) in enumerate(HGROUPS):
            g[f"kbias{gi}"] = np.tile(
                np.ascontiguousarray(kb_col[32 * h0:32 * h0 + 32 * hg]),
                (NCORES, 1))

    nc = _build(use_qbias, use_kbias, sp, half_k=compact)
    return nc, g, post, post_shard


def make_in_maps(q, k, mask, q_w, q_b, k_w, k_b):
    """Per-core input dicts + postprocess (sim / native-path use)."""
    nc, g, post, _ = _prepare(q, k, mask, q_w, q_b, k_w, k_b)
    in_maps = []
    for c in range(NCORES):
        m = {}
        for name, arr in g.items():
            rows = arr.shape[0] // NCORES
            m[name] = np.ascontiguousarray(arr[c * rows:(c + 1) * rows])
        in_maps.append(m)
    return nc, in_maps, post


# ---------------------------------------------------------------------------
# Execution: cached jit over the bass_exec custom call (axon PJRT path).
# ---------------------------------------------------------------------------

_RUNNERS = {}


def _get_runner(nc):
    key = id(nc)
    if key in _RUNNERS:
        return _RUNNERS[key]

    from concourse.bass2jax import (_bass_exec_p, install_neuronx_cc_hook,
                                    partition_id_tensor)
    from jax.sharding import Mesh, PartitionSpec, NamedSharding
    try:
        from jax.experimental.shard_map import shard_map
    except ImportError:
        shard_map = jax.shard_map

    install_neuronx_cc_hook()

    partition_name = (nc.partition_id_tensor.name
                      if nc.partition_id_tensor else None)
    in_names, out_names, out_avals = [], [], []
    for alloc in nc.m.functions[0].allocations:
        if not isinstance(alloc, mybir.MemoryLocationSet):
            continue
        name = alloc.memorylocations[0].name
        if alloc.kind == "ExternalInput":
            if name != partition_name:
                in_names.append(name)
        elif alloc.kind == "ExternalOutput":
            out_names.append(name)
            out_avals.append(jax.core.ShapedArray(
                tuple(alloc.tensor_shape), mybir.dt.np(alloc.dtype)))
    n_params = len(in_names)
    bind_names = tuple(in_names + ([partition_name] if partition_name else []))

    devices = jax.devices()[:NCORES]
    mesh = Mesh(np.asarray(devices), ("core",))
    P = PartitionSpec

    def _body(*args):
        operands = list(args)
        if partition_name:
            operands.append(partition_id_tensor())
        outs = _bass_exec_p.bind(
            *operands, out_avals=tuple(out_avals), in_names=bind_names,
            out_names=tuple(out_names), lowering_input_output_aliases=(),
            sim_require_finite=True, sim_require_nnan=True, nc=nc)
        return tuple(outs)

    sh = NamedSharding(mesh, P("core"))

    def _make_jit():
        return jax.jit(shard_map(
            _body, mesh=mesh, in_specs=(P("core"),) * n_params,
            out_specs=(P("core"),) * len(out_names), check_rep=False))

    sharded = None
    try:
        # AOT-compile with the bass effect suppressed: C++ fast-path
        # dispatch (the effectful path re-enters Python every call).
        from concourse.bass2jax import fast_dispatch_compile
        arg_structs = []
        for name in in_names:
            alloc_shape = None
            for alloc in nc.m.functions[0].allocations:
                if (isinstance(alloc, mybir.MemoryLocationSet)
                        and alloc.memorylocations[0].name == name):
                    alloc_shape = (NCORES * alloc.tensor_shape[0],
                                   *alloc.tensor_shape[1:])
                    dt = mybir.dt.np(alloc.dtype)
                    break
            arg_structs.append(
                jax.ShapeDtypeStruct(alloc_shape, dt, sharding=sh))
        sharded = fast_dispatch_compile(
            lambda: _make_jit().lower(*arg_structs).compile())
    except Exception:
        sharded = None
    if sharded is None:
        sharded = _make_jit()
    runner = {"fn": sharded, "in_names": in_names, "out_names": out_names,
              "sharding": sh, "const_cache": {}}
    _RUNNERS[key] = runner
    return runner


def _run_axon(nc, global_inputs, post_shard):
    """Dispatch via the cached jit; fetch + convert shards pipelined
    (converts hide inside the serialized tunnel transfers)."""
    from concurrent.futures import ThreadPoolExecutor
    r = _get_runner(nc)
    # device-cache the fold/unfold constants (identical every call)
    args = []
    for name in r["in_names"]:
        arr = global_inputs[name]
        if name.startswith(("fold", "unfold")):
            darr = r["const_cache"].get(name)
            if darr is None:
                darr = jax.device_put(np.asarray(arr), r["sharding"])
                r["const_cache"][name] = darr
            args.append(darr)
        else:
            args.append(arr)
    out = r["fn"](*args)[0]
    shards = sorted(out.addressable_shards,
                    key=lambda s: s.index[0].start or 0)
    final = np.empty((B, Q, NH, H, W), np.float32)

    def work(c):
        part = np.asarray(shards[c].data)
        b, qh = c // 2, c % 2
        final[b, qh * QS:(qh + 1) * QS] = post_shard(c, part)

    with ThreadPoolExecutor(4) as ex:
        list(ex.map(work, range(NCORES)))
    return final


def kernel(q, k, mask, q_w, q_b, k_w, k_b):
    import os
    nc, g, post, post_shard = _prepare(q, k, mask, q_w, q_b, k_w, k_b)
    if axon_active() and not os.environ.get("BASS_TRACE"):
        return _run_axon(nc, g, post_shard)
    from concourse.bass_utils import run_bass_kernel_spmd
    in_maps = []
    for c in range(NCORES):
        m = {}
        for name, arr in g.items():
            rows = arr.shape[0] // NCORES
            m[name] = np.ascontiguousarray(arr[c * rows:(c + 1) * rows])
        in_maps.append(m)
    res = run_bass_kernel_spmd(nc, in_maps, core_ids=list(range(NCORES)))
    out16 = np.concatenate([r["out"] for r in res.results], axis=0)
    return post(out16)
